# revision 3
# baseline (speedup 1.0000x reference)
"""Trainium2 Bass kernel, v2: TP4 x DP2 for the 4-layer decoder.

  - Cores 0-3 handle batch 0, cores 4-7 batch 1 (disjoint 4-rank
    replica groups -> the two groups' collectives run concurrently).
  - Within a group: tensor-parallel over 4 heads/core (EL=256),
    FFN 1024 hidden/core, vocab 8000 cols/core.
  - Each batch's 512 tokens split into two 256-token half-chunks;
    boundary AllReduces are per-half (512KB bf16, 4-rank Mesh) so the
    PE computes one half while the other half's AR is in flight.
  - All matmuls bf16 (residual z kept fp32 on SBUF with a bf16 shadow
    zb used as matmul rhs).  BN + biases folded host-side as before.
"""

import sys
import numpy as np

if "/opt/trn_rl_repo" not in sys.path:
    sys.path.insert(0, "/opt/trn_rl_repo")

import ml_dtypes
import concourse.bass as bass
import concourse.mybir as mybir
import concourse.tile as tile
from concourse import bacc
from concourse import bass_utils

V, D, H, L, B, S, SE = 32000, 1024, 16, 4, 2, 512, 512
DH = D // H            # 64
EPS = 1e-3
NC = 8
TPW = 4                # tensor-parallel width (per group)
HL = H // TPW          # 4 heads per core
EL = HL * DH           # 256 local head dims
FF = 4 * D
FFL = FF // TPW        # 1024 ffn hidden per core
VL = V // TPW          # 8000
VPAD = 8192
VS = VPAD // 128       # 64 vocab slices
TL = S                 # 512 tokens per group (one batch)
CH = 2                 # half-chunks
HCW = TL // CH         # 256
DT = D // 128          # 8
HT = FFL // 128        # 8
NBND = 3 * L           # 12

F32 = mybir.dt.float32
BF16 = mybir.dt.bfloat16
AF = mybir.ActivationFunctionType
OP = mybir.AluOpType

RG = [[0, 1, 2, 3], [4, 5, 6, 7]]
_CCCNT = [0]

# bias-tile column layout
COL_QKV = 0                      # L*12: l*12 + proj(6)*2 + o
COL_B1 = COL_QKV + 12 * L        # L*8: l*8 + ht
COL_SIG = COL_B1 + 8 * L         # 12*8: bnd*8 + dt
COL_BOUT = COL_SIG + 8 * NBND    # 64
COL_EPS = COL_BOUT + VS
NBCOL = COL_EPS + 1


def _build_program():
    nc = bacc.Bacc("TRN2", target_bir_lowering=False, debug=False,
                   num_devices=NC)
    dd = lambda name, shape, dtype=BF16, kind="ExternalInput": \
        nc.dram_tensor(name, shape, dtype, kind=kind).ap()

    xt = dd("xt", [D, TL], F32)
    enct = dd("enct", [D, TL])
    attw_s = dd("attw_s", [L, 128, DT * 3 * EL])    # col = dt*768+p*256+o*128
    attq_c = dd("attq_c", [L, 128, DT * EL])        # col = dt*256 + o*128
    attkv_c = dd("attkv_c", [L, 128, DT * 2 * EL])  # col = dt*512+{k0,v256}+o*128
    wo_s = dd("wo_s", [L, 128, 2 * D])              # col = kt*1024 + dout
    wo_c = dd("wo_c", [L, 128, 2 * D])
    w1p = dd("w1p", [L, 128, DT * FFL])             # col = dt*1024 + f
    w2p = dd("w2p", [L, 128, HT * D])               # col = ht*1024 + dout
    woutp = dd("woutp", [128, VS * D])              # col = vs*1024 + dt*128 + j
    biasp = dd("biasp", [128, NBCOL], F32)
    maskd = dd("maskd", [128, 128])                 # strictly-lower 0/1
    identd = dd("identd", [128, 128])
    onesd = dd("onesd", [128, 64])
    logt = dd("logt", [VPAD, TL], F32, kind="ExternalOutput")

    from contextlib import ExitStack
    with tile.TileContext(nc) as tc, ExitStack() as _es:
        P = lambda **kw: _es.enter_context(tc.tile_pool(**kw))
        cst = P(name="cst", bufs=1)
        zp = P(name="zp", bufs=1)
        zbp = P(name="zbp", bufs=1)
        encp = P(name="encp", bufs=1)
        qp = P(name="qp", bufs=2)
        kvp = P(name="kvp", bufs=2)
        ckvp = P(name="ckvp", bufs=2)
        vap = P(name="vap", bufs=2)
        cvap = P(name="cvap", bufs=2)
        esp = P(name="esp", bufs=5)
        hdp = P(name="hdp", bufs=2)
        csp = P(name="csp", bufs=2)
        hfp = P(name="hfp", bufs=2)
        arp = P(name="arp", bufs=2)
        aop = P(name="aop", bufs=2)
        wap = P(name="wap", bufs=2)
        waqc = P(name="waqc", bufs=1)
        wakv = P(name="wakv", bufs=1)
        wop = P(name="wop", bufs=1)
        w1pool = P(name="w1pool", bufs=1)
        w2pool = P(name="w2pool", bufs=1)
        wvp = P(name="wvp", bufs=3)
        osp = P(name="osp", bufs=3)
        ps = P(name="ps", bufs=8, space="PSUM")
        dram = P(name="dram", bufs=4, space="DRAM")

        bias_sb = cst.tile([128, NBCOL], F32)
        nc.sync.dma_start(bias_sb[:], biasp[:])
        mask_sb = cst.tile([128, 128], BF16)
        nc.sync.dma_start(mask_sb[:], maskd[:])
        ident = cst.tile([128, 128], BF16)
        nc.sync.dma_start(ident[:], identd[:])
        ones_sb = cst.tile([128, 64], BF16)
        nc.sync.dma_start(ones_sb[:], onesd[:])
        zeros_sb = cst.tile([128, 256], BF16)
        nc.vector.tensor_scalar_mul(zeros_sb[:, 0:64], ones_sb[:], 0.0)
        nc.vector.tensor_copy(zeros_sb[:, 64:128], zeros_sb[:, 0:64])
        nc.vector.tensor_copy(zeros_sb[:, 128:256], zeros_sb[:, 0:128])

        encs = []
        for dt in range(DT):
            et = encp.tile([128, TL], BF16, name=f"enc{dt}")
            nc.sync.dma_start(et[:], enct[dt * 128:(dt + 1) * 128, :])
            encs.append(et)

        z = [None] * DT
        zb = [None] * DT
        for dt in range(DT):
            zt = zp.tile([128, TL], F32, name=f"z{dt}")
            nc.sync.dma_start(zt[:], xt[dt * 128:(dt + 1) * 128, :])
            z[dt] = zt
            zbt = zbp.tile([128, TL], BF16, name=f"zb{dt}")
            if dt % 2 == 0:
                nc.scalar.activation(zbt[:], zt[:], AF.Copy)
            else:
                nc.vector.tensor_copy(zbt[:], zt[:])
            zb[dt] = zbt

        def bcol(c):
            return bias_sb[:, c:c + 1]

        def hc(ch):
            return slice(ch * HCW, (ch + 1) * HCW)

        def build_vaug(kb, pool, vsrc, vwsl):
            """Token-major vaug [128, 384] built straight from the v
            projection: partitions = tokens of key-block kb, cols =
            [h0|ones|h1|h2|ones|h3].  vsrc(dt) is the [128, 128] lhsT
            slice of the activation source for this kb; vwsl(dt, o) the
            [128, 128] v-weight slice for feature half o."""
            va = pool.tile([128, 384], BF16, name=f"va{kb}")
            for o in range(2):
                ppf = ps.tile([128, TL], F32, name="pvt", tag="mm")
                pp = ppf[:, 0:128]
                for dt in range(DT):
                    nc.tensor.matmul(pp[:], vsrc(dt), vwsl(dt, o),
                                     start=(dt == 0), stop=(dt == DT - 1))
                base = 192 * o
                nc.scalar.activation(va[:, base:base + 64], pp[:, 0:64],
                                     AF.Identity)
                dsth1 = va[:, 128:192] if o == 0 else va[:, 320:384]
                nc.scalar.activation(dsth1, pp[:, 64:128], AF.Identity)
            nc.vector.tensor_copy(va[:, 64:128], ones_sb[:])
            nc.vector.tensor_copy(va[:, 256:320], ones_sb[:])
            return va

        def attn_core(qt, kt, vaug, ch, causal, bvc):
            """qt: 2x[128,HCW]; kt: 2x[128,TL]; vaug: list of [128,384].
            bvc: bias column base for the deferred v bias (+o).
            Returns hd: 2 tiles [128, HCW] bf16."""
            nkb = 2 * (ch + 1) if causal else 4
            hd = [hdp.tile([128, HCW], BF16, name=f"hd{o}") for o in range(2)]
            for h in range(HL):
                o, ro = h // 2, (h % 2) * 64
                es = []
                for kb in range(nkb):
                    q0 = max(0, kb * 128 - ch * HCW) if causal else 0
                    ppf = ps.tile([128, TL], F32, name="psc", tag="mm")
                    pp = ppf[:, 0:HCW]
                    nc.tensor.matmul(pp[:],
                                     kt[o][ro:ro + 64,
                                           kb * 128:(kb + 1) * 128],
                                     qt[o][ro:ro + 64, :],
                                     start=True, stop=True)
                    et = esp.tile([128, HCW], BF16, name="es")
                    nc.scalar.activation(et[:, q0:HCW], pp[:, q0:HCW], AF.Exp)
                    if causal and kb * 128 >= ch * HCW:
                        if q0 > 0:
                            nc.vector.tensor_copy(et[:, 0:q0],
                                                  zeros_sb[:, 0:q0])
                        nc.vector.tensor_tensor(et[:, q0:q0 + 128],
                                                et[:, q0:q0 + 128],
                                                mask_sb[:], op=OP.mult)
                    es.append(et)
                pof = ps.tile([128, TL], F32, name="po", tag="mm")
                po = pof[:, 0:HCW]
                # stationary window: [val|ones] for even h, [ones|val] odd
                ws = 64 * h + (64 if h >= 2 else 0)
                for kb in range(nkb):
                    nc.tensor.matmul(po[:], vaug[kb][:, ws:ws + 128],
                                     es[kb][:],
                                     start=(kb == 0), stop=(kb == nkb - 1))
                if h % 2 == 0:
                    nrows, crows = po[0:64, :], po[64:128, :]
                else:
                    nrows, crows = po[64:128, :], po[0:64, :]
                cs = csp.tile([64, HCW], F32, name="cs")
                nc.scalar.activation(cs[:], crows, AF.Identity,
                                     bias=bias_sb[0:64, COL_EPS:COL_EPS + 1])
                rc = csp.tile([64, HCW], F32, name="rc")
                nc.vector.reciprocal_approx_fast(out=rc[:], in_=cs[:])
                cs2 = csp.tile([64, HCW], F32, name="cs2")
                nc.vector.tensor_tensor(cs2[:], nrows, rc[:], op=OP.mult)
                nc.scalar.activation(
                    hd[o][ro:ro + 64, :], cs2[:], AF.Identity,
                    bias=bias_sb[ro:ro + 64, bvc + o:bvc + o + 1])
            if causal and ch == 0:
                # position 0 attends to nothing -> exactly zero output
                for o in range(2):
                    nc.vector.tensor_scalar_mul(hd[o][:, 0:1],
                                                hd[o][:, 0:1], 0.0)
            return hd

        def partial_ar(src, wsel, mode, nsrc):
            """src: list of tiles [128, HCW] (contraction tiles);
            wsel cols: i*1024 + dout*128.  AllReduce [D, HCW] bf16."""
            arin = dram.tile([D, HCW], BF16, name="arin")
            arout = dram.tile([D, HCW], BF16, name="arout")
            ocw = aop.tile([128, DT * HCW], BF16, name="ocw")
            for half in range(2):
                for dout in range(half * 4, half * 4 + 4):
                    pwf = ps.tile([128, TL], F32, name="pw", tag="mm")
                    pw = pwf[:, 0:HCW]
                    for i in range(nsrc):
                        c0 = i * D + dout * 128
                        nc.tensor.matmul(pw[:], wsel[:, c0:c0 + 128],
                                         src[i][:],
                                         start=(i == 0), stop=(i == nsrc - 1))
                    osl = ocw[:, dout * HCW:(dout + 1) * HCW]
                    if dout % 2 == 0:
                        nc.scalar.activation(osl, pw[:], AF.Copy)
                    else:
                        nc.vector.tensor_copy(osl, pw[:])
                h0 = half * 4
                nc.sync.dma_start(
                    arin[h0 * 128:(h0 + 4) * 128, :].rearrange(
                        "(dt p) t -> p dt t", p=128),
                    ocw[:, h0 * HCW:(h0 + 4) * HCW].rearrange(
                        "p (dt t) -> p dt t", t=HCW))
            import os as _os
            nlim = int(_os.environ.get("K2_CC_N", "999"))
            _CCCNT[0] += 1
            if _os.environ.get("K2_SKIP_CC") == "1" or _CCCNT[0] > nlim:
                return arin
            rg = [[0, 1, 2, 3, 4, 5, 6, 7]] \
                if _os.environ.get("K2_RG8") == "1" else RG
            nc.gpsimd.collective_compute("AllReduce", OP.add,
                                         replica_groups=rg,
                                         ins=[arin[:]], outs=[arout[:]])
            return arout

        def boundary(ch, arout, bnd):
            art = arp.tile([128, DT * HCW], BF16, name="art")
            for half in range(2):
                h0 = half * 4
                nc.sync.dma_start(
                    art[:, h0 * HCW:(h0 + 4) * HCW].rearrange(
                        "p (dt t) -> p dt t", t=HCW),
                    arout[h0 * 128:(h0 + 4) * 128, :].rearrange(
                        "(dt p) t -> p dt t", p=128))
            for dt in range(DT):
                # bf16 shadow on the critical path (DVE), fp32 residual
                # update off-path on the otherwise-idle GpSimd engine
                nc.vector.scalar_tensor_tensor(
                    zb[dt][:, hc(ch)], z[dt][:, hc(ch)],
                    bcol(COL_SIG + bnd * 8 + dt),
                    art[:, dt * HCW:(dt + 1) * HCW], OP.mult, OP.add)
                nc.vector.scalar_tensor_tensor(
                    z[dt][:, hc(ch)], z[dt][:, hc(ch)],
                    bcol(COL_SIG + bnd * 8 + dt),
                    art[:, dt * HCW:(dt + 1) * HCW], OP.mult, OP.add)

        def proj2(srcs, wsl, bc0, nm, pool):
            """2 out-tiles [128, HCW] = w.T @ src (+bias)."""
            outs = []
            for o in range(2):
                ppf = ps.tile([128, TL], F32, name="pp", tag="mm")
                pp = ppf[:, 0:HCW]
                for dt in range(DT):
                    nc.tensor.matmul(pp[:], wsl(dt, o), srcs(dt),
                                     start=(dt == 0), stop=(dt == DT - 1))
                pt = pool.tile([128, HCW], BF16, name=f"{nm}{o}")
                nc.scalar.activation(pt[:], pp[:], AF.Identity,
                                     bias=bcol(bc0 + o))
                outs.append(pt)
            return outs

        for l in range(L):
            aw_s = wap.tile([128, DT * 3 * EL], BF16, name="aw")
            nc.sync.dma_start(aw_s[:], attw_s[l])
            wo_s_t = wop.tile([128, 2 * D], BF16, name="wot")
            nc.sync.dma_start(wo_s_t[:], wo_s[l])
            aq_c = waqc.tile([128, DT * EL], BF16, name="aqc")
            nc.sync.dma_start(aq_c[:], attq_c[l])
            akv_c = wakv.tile([128, DT * 2 * EL], BF16, name="akvc")
            nc.sync.dma_start(akv_c[:], attkv_c[l])

            # self attention per half-chunk
            k_t = [kvp.tile([128, TL], BF16, name=f"kt{o}") for o in range(2)]
            vaug = [None] * 4
            ars = []
            for ch in range(CH):
                q_t = []
                for o in range(2):
                    ppf = ps.tile([128, TL], F32, name="pq", tag="mm")
                    pp = ppf[:, 0:HCW]
                    for dt in range(DT):
                        c0 = dt * 768 + 0 * 256 + o * 128
                        nc.tensor.matmul(pp[:], aw_s[:, c0:c0 + 128],
                                         zb[dt][:, hc(ch)],
                                         start=(dt == 0), stop=(dt == DT - 1))
                    qt = qp.tile([128, HCW], BF16, name=f"qt{o}")
                    nc.scalar.activation(qt[:], pp[:], AF.Identity,
                                         bias=bcol(COL_QKV + l * 12 + o))
                    q_t.append(qt)
                for o in range(2):
                    ppf = ps.tile([128, TL], F32, name="pkv", tag="mm")
                    pp = ppf[:, 0:HCW]
                    for dt in range(DT):
                        c0 = dt * 768 + 256 + o * 128
                        nc.tensor.matmul(pp[:], aw_s[:, c0:c0 + 128],
                                         zb[dt][:, hc(ch)],
                                         start=(dt == 0),
                                         stop=(dt == DT - 1))
                    nc.scalar.activation(
                        k_t[o][:, hc(ch)], pp[:], AF.Identity,
                        bias=bcol(COL_QKV + l * 12 + 2 + o))
                for kb in range(2 * ch, 2 * ch + 2):
                    vaug[kb] = build_vaug(
                        kb, vap,
                        lambda dt, kb=kb: zb[dt][:, kb * 128:(kb + 1) * 128],
                        lambda dt, o: aw_s[:, dt * 768 + 512 + o * 128:
                                           dt * 768 + 512 + o * 128 + 128])
                hd = attn_core(q_t, k_t, vaug, ch, True,
                               COL_QKV + l * 12 + 4)
                ars.append(partial_ar(hd, wo_s_t, "wo", 2))

            # hoisted cross k/v (full 512 tokens, encoder-sourced)
            ck_t = [ckvp.tile([128, TL], BF16, name=f"ck{o}")
                    for o in range(2)]
            for o in range(2):
                pp = ps.tile([128, TL], F32, name="pck", tag="mm")
                for dt in range(DT):
                    c0 = dt * 512 + o * 128
                    nc.tensor.matmul(pp[:], akv_c[:, c0:c0 + 128],
                                     encs[dt][:],
                                     start=(dt == 0), stop=(dt == DT - 1))
                nc.scalar.activation(
                    ck_t[o][:], pp[:], AF.Identity,
                    bias=bcol(COL_QKV + l * 12 + 8 + o))
            cvaug = [build_vaug(
                kb, cvap,
                lambda dt, kb=kb: encs[dt][:, kb * 128:(kb + 1) * 128],
                lambda dt, o: akv_c[:, dt * 512 + 256 + o * 128:
                                    dt * 512 + 256 + o * 128 + 128])
                for kb in range(4)]

            wo_c_t = wop.tile([128, 2 * D], BF16, name="woc")
            nc.sync.dma_start(wo_c_t[:], wo_c[l])
            w1t = w1pool.tile([128, DT * FFL], BF16, name="w1t")
            nc.sync.dma_start(w1t[:], w1p[l])
            w2t = w2pool.tile([128, HT * D], BF16, name="w2t")
            nc.sync.dma_start(w2t[:], w2p[l])
            arc = []
            for ch in range(CH):
                boundary(ch, ars[ch], 3 * l)
                zsrc = lambda dt, ch=ch: zb[dt][:, hc(ch)]
                q_c = proj2(zsrc,
                            lambda dt, o: aq_c[:, dt * 256 + o * 128:
                                               dt * 256 + o * 128 + 128],
                            COL_QKV + l * 12 + 6, "qc", qp)
                hd = attn_core(q_c, ck_t, cvaug, ch, False,
                               COL_QKV + l * 12 + 10)
                arc.append(partial_ar(hd, wo_c_t, "wo", 2))

            # FFN per half-chunk
            arf = []
            for ch in range(CH):
                boundary(ch, arc[ch], 3 * l + 1)
                hts = []
                for ht in range(HT):
                    ppf = ps.tile([128, TL], F32, name="pf", tag="mm")
                    pp = ppf[:, 0:HCW]
                    for dt in range(DT):
                        c0 = dt * FFL + ht * 128
                        nc.tensor.matmul(pp[:], w1t[:, c0:c0 + 128],
                                         zb[dt][:, hc(ch)],
                                         start=(dt == 0), stop=(dt == DT - 1))
                    htile = hfp.tile([128, HCW], BF16, name=f"hf{ht}")
                    nc.scalar.activation(htile[:], pp[:], AF.Relu,
                                         bias=bcol(COL_B1 + l * 8 + ht))
                    hts.append(htile)
                arf.append(partial_ar(hts, w2t, "ffn2", HT))
            for ch in range(CH):
                boundary(ch, arf[ch], 3 * l + 2)

        # vocab projection: first NCHUNKED slices run on half-chunk 0
        # only (early start under the final ARs), then their half 1, then
        # the remainder full-width N=512
        NCHUNKED = 12

        def vocab_slice(vs, cols, wt):
            w = cols.stop - cols.start
            ppf = ps.tile([128, TL], F32, name="pv", tag="mm")
            pp = ppf[:, 0:w]
            for dt in range(DT):
                nc.tensor.matmul(pp[:], wt[:, dt * 128:(dt + 1) * 128],
                                 zb[dt][:, cols],
                                 start=(dt == 0), stop=(dt == DT - 1))
            osb = osp.tile([128, TL], F32, name="osb")
            osl = osb[:, 0:w]
            if vs % 2 == 0:
                nc.scalar.activation(osl, pp[:], AF.Identity,
                                     bias=bcol(COL_BOUT + vs))
            else:
                nc.vector.tensor_scalar_add(osl, pp[:], bcol(COL_BOUT + vs))
            nc.sync.dma_start(logt[vs * 128:(vs + 1) * 128, cols], osl)

        for ph, (v0, v1, cols) in enumerate([
                (0, NCHUNKED, hc(0)), (0, NCHUNKED, hc(1)),
                (NCHUNKED, VS, slice(0, TL))]):
            for vs in range(v0, v1):
                wt = wvp.tile([128, D], BF16, name="wv")
                nc.sync.dma_start(wt[:], woutp[:, vs * D:(vs + 1) * D])
                vocab_slice(vs, cols, wt)
    nc.compile()
    return nc


def _host_prepare(inputs):
    f = lambda a: np.asarray(a, dtype=np.float64)
    tobf = lambda a: a.astype(ml_dtypes.bfloat16)
    seq = np.asarray(inputs["sequence"])
    emb = np.asarray(inputs["emb"], dtype=np.float32)
    pes = np.asarray(inputs["pes"], dtype=np.float32)
    enc = np.asarray(inputs["encoder_out"], dtype=np.float32)

    x0 = emb[seq] + pes[None, :, :]                   # [B, S, D]
    xts = [np.ascontiguousarray(x0[b].T.astype(np.float32))
           for b in range(B)]                         # [D, S] per batch
    encts = [np.ascontiguousarray(tobf(enc[b].T)) for b in range(B)]

    mask = (np.arange(128)[:, None] < np.arange(128)[None, :])
    maskd = np.ascontiguousarray(tobf(mask.astype(np.float32)))

    bf = ml_dtypes.bfloat16
    attw_s = np.zeros((TPW, L, 128, DT * 3 * EL), bf)
    attq_c = np.zeros((TPW, L, 128, DT * EL), bf)
    attkv_c = np.zeros((TPW, L, 128, DT * 2 * EL), bf)
    wo_s_p = np.zeros((TPW, L, 128, 2 * D), bf)
    wo_c_p = np.zeros((TPW, L, 128, 2 * D), bf)
    w1pp = np.zeros((TPW, L, 128, DT * FFL), bf)
    w2pp = np.zeros((TPW, L, 128, HT * D), bf)
    woutpp = np.zeros((TPW, 128, VS * D), bf)
    biaspp = np.zeros((TPW, 128, NBCOL), np.float32)

    def pack_kxm(w, ncols):
        kt = w.shape[0] // 128
        return w.reshape(kt, 128, ncols).transpose(1, 0, 2).reshape(
            128, kt * ncols)

    sig = np.ones(D)
    gam = np.zeros(D)
    for l in range(L):
        for which, (wq, bq, wk, bk, wv, bv, wo, bo, g, be, m, v) in enumerate([
            (inputs["wq_s"][l], inputs["bq_s"][l], inputs["wk_s"][l],
             inputs["bk_s"][l], inputs["wv_s"][l], inputs["bv_s"][l],
             inputs["wo_s"][l], inputs["bo_s"][l], inputs["g1"][l],
             inputs["be1"][l], inputs["m1"][l], inputs["v1"][l]),
            (inputs["wq_c"][l], inputs["bq_c"][l], inputs["wk_c"][l],
             inputs["bk_c"][l], inputs["wv_c"][l], inputs["bv_c"][l],
             inputs["wo_c"][l], inputs["bo_c"][l], inputs["g2"][l],
             inputs["be2"][l], inputs["m2"][l], inputs["v2"][l]),
        ]):
            wq, wk, wv = f(wq), f(wk), f(wv)          # [H, D, DH]
            bq, bk, bv = f(bq), f(bk), f(bv)          # [H, DH]
            wo, bo = f(wo), f(bo)
            for r in range(TPW):
                h0 = r * HL
                wql = wq[h0:h0 + HL].transpose(1, 0, 2).reshape(D, EL)
                wkl = wk[h0:h0 + HL].transpose(1, 0, 2).reshape(D, EL)
                wvl = wv[h0:h0 + HL].transpose(1, 0, 2).reshape(D, EL)
                bql = bq[h0:h0 + HL].reshape(EL)
                bkl = bk[h0:h0 + HL].reshape(EL)
                bvl = bv[h0:h0 + HL].reshape(EL)
                wq_eff = (sig[:, None] * wql) / 8.0
                bq_eff = (gam @ wql + bql) / 8.0
                if which == 0:
                    wk_eff = sig[:, None] * wkl
                    bk_eff = gam @ wkl + bkl
                    wv_eff = sig[:, None] * wvl
                    bv_eff = gam @ wvl + bvl
                    wcat = np.concatenate([wq_eff, wk_eff, wv_eff], axis=1)
                    attw_s[r, l] = tobf(
                        pack_kxm(wcat, 3 * EL).astype(np.float32))
                    cb = COL_QKV + l * 12
                    for o in range(2):
                        biaspp[r, :, cb + 0 + o] = \
                            bq_eff[o * 128:(o + 1) * 128]
                        biaspp[r, :, cb + 2 + o] = \
                            bk_eff[o * 128:(o + 1) * 128]
                        biaspp[r, :, cb + 4 + o] = \
                            bv_eff[o * 128:(o + 1) * 128]
                else:
                    attq_c[r, l] = tobf(
                        pack_kxm(wq_eff, EL).astype(np.float32))
                    kvcat = np.concatenate([wkl, wvl], axis=1)
                    attkv_c[r, l] = tobf(
                        pack_kxm(kvcat, 2 * EL).astype(np.float32))
                    cb = COL_QKV + l * 12
                    for o in range(2):
                        biaspp[r, :, cb + 6 + o] = \
                            bq_eff[o * 128:(o + 1) * 128]
                        biaspp[r, :, cb + 8 + o] = bkl[o * 128:(o + 1) * 128]
                        biaspp[r, :, cb + 10 + o] = bvl[o * 128:(o + 1) * 128]
                wo_loc = wo[r * EL:(r + 1) * EL, :]       # [256, 1024]
                wo_pk = wo_loc.reshape(2, 128, D).transpose(1, 0, 2).reshape(
                    128, 2 * D)
                (wo_s_p if which == 0 else wo_c_p)[r, l] = tobf(
                    wo_pk.astype(np.float32))
            bnd = 3 * l + which
            for r in range(TPW):
                for dt in range(DT):
                    biaspp[r, :, COL_SIG + bnd * 8 + dt] = \
                        sig[dt * 128:(dt + 1) * 128].astype(np.float32)
            beta = gam + bo
            s = f(g) / np.sqrt(f(v) + EPS)
            cshift = f(be) - f(m) * s
            sig = s
            gam = s * beta + cshift

        w1, b1 = f(inputs["w1"][l]), f(inputs["b1"][l])
        w2, b2 = f(inputs["w2"][l]), f(inputs["b2"][l])
        for r in range(TPW):
            cols = slice(r * FFL, (r + 1) * FFL)
            w1_eff = sig[:, None] * w1[:, cols]
            b1_eff = gam @ w1[:, cols] + b1[cols]
            w1pp[r, l] = tobf(pack_kxm(w1_eff, FFL).astype(np.float32))
            w2pp[r, l] = tobf(pack_kxm(w2[cols, :], D).astype(np.float32))
            for ht in range(HT):
                biaspp[r, :, COL_B1 + l * 8 + ht] = \
                    b1_eff[ht * 128:(ht + 1) * 128].astype(np.float32)
        bnd = 3 * l + 2
        for r in range(TPW):
            for dt in range(DT):
                biaspp[r, :, COL_SIG + bnd * 8 + dt] = \
                    sig[dt * 128:(dt + 1) * 128].astype(np.float32)
        beta = gam + b2
        s = f(inputs["g3"][l]) / np.sqrt(f(inputs["v3"][l]) + EPS)
        cshift = f(inputs["be3"][l]) - f(inputs["m3"][l]) * s
        sig = s
        gam = s * beta + cshift

    wout, bout = f(inputs["w_out"]), f(inputs["b_out"])
    for r in range(TPW):
        wsl = np.zeros((D, VPAD))
        bsl = np.zeros(VPAD)
        cols = slice(r * VL, (r + 1) * VL)
        wsl[:, :VL] = wout[:, cols]
        bsl[:VL] = bout[cols]
        wout_eff = sig[:, None] * wsl
        bout_eff = gam @ wsl + bsl
        woutpp[r] = tobf(wout_eff.reshape(DT, 128, VS, 128).transpose(
            1, 2, 0, 3).reshape(128, VS * D).astype(np.float32))
        for vs in range(VS):
            biaspp[r, :, COL_BOUT + vs] = \
                bout_eff[vs * 128:(vs + 1) * 128].astype(np.float32)

    biaspp[:, :, COL_EPS] = 1e-30
    in_maps = []
    for c in range(NC):
        g, r = c // TPW, c % TPW
        in_maps.append({
            "xt": xts[g], "enct": encts[g],
            "attw_s": attw_s[r], "attq_c": attq_c[r], "attkv_c": attkv_c[r],
            "wo_s": wo_s_p[r], "wo_c": wo_c_p[r],
            "w1p": w1pp[r], "w2p": w2pp[r], "woutp": woutpp[r],
            "biasp": biaspp[r], "maskd": maskd,
            "identd": tobf(np.eye(128, dtype=np.float32)),
            "onesd": np.ones((128, 64), dtype=ml_dtypes.bfloat16),
        })
    return in_maps


_NC_CACHE = {}


def _get_program():
    if "nc" not in _NC_CACHE:
        _NC_CACHE["nc"] = _build_program()
    return _NC_CACHE["nc"]


def run(inputs, trace=False):
    nc = _get_program()
    in_maps = _host_prepare(inputs)
    res = bass_utils.run_bass_kernel_spmd(nc, in_maps, list(range(NC)),
                                          trace=trace)
    out = np.empty((B, S, V), np.float32)
    for c in range(NC):
        g, r = c // TPW, c % TPW
        out[g, :, r * VL:(r + 1) * VL] = res.results[c]["logt"][:VL, :].T
    return out, res


def kernel(**inputs):
    out, _ = run(inputs)
    return out


# revision 4
# speedup vs baseline: 1.0169x; 1.0169x over previous
"""Trainium2 Bass kernel, v2: TP4 x DP2 for the 4-layer decoder.

  - Cores 0-3 handle batch 0, cores 4-7 batch 1 (disjoint 4-rank
    replica groups -> the two groups' collectives run concurrently).
  - Within a group: tensor-parallel over 4 heads/core (EL=256),
    FFN 1024 hidden/core, vocab 8000 cols/core.
  - Each batch's 512 tokens split into two 256-token half-chunks;
    boundary AllReduces are per-half (512KB bf16, 4-rank Mesh) so the
    PE computes one half while the other half's AR is in flight.
  - All matmuls bf16 (residual z kept fp32 on SBUF with a bf16 shadow
    zb used as matmul rhs).  BN + biases folded host-side as before.
"""

import sys
import numpy as np

if "/opt/trn_rl_repo" not in sys.path:
    sys.path.insert(0, "/opt/trn_rl_repo")

import ml_dtypes
import concourse.bass as bass
import concourse.mybir as mybir
import concourse.tile as tile
from concourse import bacc
from concourse import bass_utils

V, D, H, L, B, S, SE = 32000, 1024, 16, 4, 2, 512, 512
DH = D // H            # 64
EPS = 1e-3
NC = 8
TPW = 4                # tensor-parallel width (per group)
HL = H // TPW          # 4 heads per core
EL = HL * DH           # 256 local head dims
FF = 4 * D
FFL = FF // TPW        # 1024 ffn hidden per core
VL = V // TPW          # 8000
VPAD = 8192
VS = VPAD // 128       # 64 vocab slices
TL = S                 # 512 tokens per group (one batch)
CH = 2                 # half-chunks
HCW = TL // CH         # 256
DT = D // 128          # 8
HT = FFL // 128        # 8
NBND = 3 * L           # 12

F32 = mybir.dt.float32
BF16 = mybir.dt.bfloat16
AF = mybir.ActivationFunctionType
OP = mybir.AluOpType

RG = [[0, 1, 2, 3], [4, 5, 6, 7]]
_CCCNT = [0]

# bias-tile column layout
COL_QKV = 0                      # L*12: l*12 + proj(6)*2 + o
COL_B1 = COL_QKV + 12 * L        # L*8: l*8 + ht
COL_SIG = COL_B1 + 8 * L         # 12*8: bnd*8 + dt
COL_BOUT = COL_SIG + 8 * NBND    # 64
COL_EPS = COL_BOUT + VS
NBCOL = COL_EPS + 1


def _build_program():
    nc = bacc.Bacc("TRN2", target_bir_lowering=False, debug=False,
                   num_devices=NC)
    dd = lambda name, shape, dtype=BF16, kind="ExternalInput": \
        nc.dram_tensor(name, shape, dtype, kind=kind).ap()

    xt = dd("xt", [D, TL], F32)
    enct = dd("enct", [D, TL])
    attw_s = dd("attw_s", [L, 128, DT * 3 * EL])    # col = dt*768+p*256+o*128
    attq_c = dd("attq_c", [L, 128, DT * EL])        # col = dt*256 + o*128
    attkv_c = dd("attkv_c", [L, 128, DT * 2 * EL])  # col = dt*512+{k0,v256}+o*128
    wo_s = dd("wo_s", [L, 128, 2 * D])              # col = kt*1024 + dout
    wo_c = dd("wo_c", [L, 128, 2 * D])
    w1p = dd("w1p", [L, 128, DT * FFL])             # col = dt*1024 + f
    w2p = dd("w2p", [L, 128, HT * D])               # col = ht*1024 + dout
    woutp = dd("woutp", [128, VS * D])              # col = vs*1024 + dt*128 + j
    biasp = dd("biasp", [128, NBCOL], F32)
    maskd = dd("maskd", [128, 128])                 # strictly-lower 0/1
    identd = dd("identd", [128, 128])
    onesd = dd("onesd", [128, 64])
    logt = dd("logt", [VPAD, TL], F32, kind="ExternalOutput")

    from contextlib import ExitStack
    with tile.TileContext(nc) as tc, ExitStack() as _es:
        P = lambda **kw: _es.enter_context(tc.tile_pool(**kw))
        cst = P(name="cst", bufs=1)
        zp = P(name="zp", bufs=1)
        zbp = P(name="zbp", bufs=1)
        encp = P(name="encp", bufs=1)
        qp = P(name="qp", bufs=2)
        kvp = P(name="kvp", bufs=2)
        ckvp = P(name="ckvp", bufs=2)
        vap = P(name="vap", bufs=2)
        cvap = P(name="cvap", bufs=2)
        esp = P(name="esp", bufs=5)
        hdp = P(name="hdp", bufs=2)
        csp = P(name="csp", bufs=2)
        hfp = P(name="hfp", bufs=2)
        arp = P(name="arp", bufs=2)
        aop = P(name="aop", bufs=2)
        wap = P(name="wap", bufs=2)
        waqc = P(name="waqc", bufs=1)
        wakv = P(name="wakv", bufs=1)
        wop = P(name="wop", bufs=1)
        w1pool = P(name="w1pool", bufs=1)
        w2pool = P(name="w2pool", bufs=1)
        wvp = P(name="wvp", bufs=3)
        osp = P(name="osp", bufs=3)
        ps = P(name="ps", bufs=8, space="PSUM")
        dram = P(name="dram", bufs=4, space="DRAM")

        bias_sb = cst.tile([128, NBCOL], F32)
        nc.sync.dma_start(bias_sb[:], biasp[:])
        mask_sb = cst.tile([128, 128], BF16)
        nc.sync.dma_start(mask_sb[:], maskd[:])
        ident = cst.tile([128, 128], BF16)
        nc.sync.dma_start(ident[:], identd[:])
        ones_sb = cst.tile([128, 64], BF16)
        nc.sync.dma_start(ones_sb[:], onesd[:])
        zeros_sb = cst.tile([128, 256], BF16)
        nc.vector.tensor_scalar_mul(zeros_sb[:, 0:64], ones_sb[:], 0.0)
        nc.vector.tensor_copy(zeros_sb[:, 64:128], zeros_sb[:, 0:64])
        nc.vector.tensor_copy(zeros_sb[:, 128:256], zeros_sb[:, 0:128])

        z = [None] * DT
        zb = [None] * DT
        for dt in range(DT):
            zt = zp.tile([128, TL], F32, name=f"z{dt}")
            nc.sync.dma_start(zt[:], xt[dt * 128:(dt + 1) * 128, :])
            z[dt] = zt
            zbt = zbp.tile([128, TL], BF16, name=f"zb{dt}")
            if dt % 2 == 0:
                nc.scalar.activation(zbt[:], zt[:], AF.Copy)
            else:
                nc.vector.tensor_copy(zbt[:], zt[:])
            zb[dt] = zbt

        # encoder activations loaded after z (needed only at cross-attn)
        encs = []
        for dt in range(DT):
            et = encp.tile([128, TL], BF16, name=f"enc{dt}")
            nc.sync.dma_start(et[:], enct[dt * 128:(dt + 1) * 128, :])
            encs.append(et)

        def bcol(c):
            return bias_sb[:, c:c + 1]

        def hc(ch):
            return slice(ch * HCW, (ch + 1) * HCW)

        def build_vaug(kb, pool, vsrc, vwsl):
            """Token-major vaug [128, 384] built straight from the v
            projection: partitions = tokens of key-block kb, cols =
            [h0|ones|h1|h2|ones|h3].  vsrc(dt) is the [128, 128] lhsT
            slice of the activation source for this kb; vwsl(dt, o) the
            [128, 128] v-weight slice for feature half o."""
            va = pool.tile([128, 384], BF16, name=f"va{kb}")
            for o in range(2):
                ppf = ps.tile([128, TL], F32, name="pvt", tag="mm")
                pp = ppf[:, 0:128]
                for dt in range(DT):
                    nc.tensor.matmul(pp[:], vsrc(dt), vwsl(dt, o),
                                     start=(dt == 0), stop=(dt == DT - 1))
                base = 192 * o
                nc.scalar.activation(va[:, base:base + 64], pp[:, 0:64],
                                     AF.Identity)
                dsth1 = va[:, 128:192] if o == 0 else va[:, 320:384]
                nc.scalar.activation(dsth1, pp[:, 64:128], AF.Identity)
            nc.vector.tensor_copy(va[:, 64:128], ones_sb[:])
            nc.vector.tensor_copy(va[:, 256:320], ones_sb[:])
            return va

        def attn_core(qt, kt, vaug, ch, causal, bvc):
            """qt: 2x[128,HCW]; kt: 2x[128,TL]; vaug: list of [128,384].
            bvc: bias column base for the deferred v bias (+o).
            Returns hd: 2 tiles [128, HCW] bf16."""
            nkb = 2 * (ch + 1) if causal else 4
            hd = [hdp.tile([128, HCW], BF16, name=f"hd{o}") for o in range(2)]
            for h in range(HL):
                o, ro = h // 2, (h % 2) * 64
                es = []
                for kb in range(nkb):
                    q0 = max(0, kb * 128 - ch * HCW) if causal else 0
                    ppf = ps.tile([128, TL], F32, name="psc", tag="mm")
                    pp = ppf[:, 0:HCW]
                    nc.tensor.matmul(pp[:],
                                     kt[o][ro:ro + 64,
                                           kb * 128:(kb + 1) * 128],
                                     qt[o][ro:ro + 64, :],
                                     start=True, stop=True)
                    et = esp.tile([128, HCW], BF16, name="es")
                    nc.scalar.activation(et[:, q0:HCW], pp[:, q0:HCW], AF.Exp)
                    if causal and kb * 128 >= ch * HCW:
                        if q0 > 0:
                            nc.vector.tensor_copy(et[:, 0:q0],
                                                  zeros_sb[:, 0:q0])
                        nc.vector.tensor_tensor(et[:, q0:q0 + 128],
                                                et[:, q0:q0 + 128],
                                                mask_sb[:], op=OP.mult)
                    es.append(et)
                pof = ps.tile([128, TL], F32, name="po", tag="mm")
                po = pof[:, 0:HCW]
                # stationary window: [val|ones] for even h, [ones|val] odd
                ws = 64 * h + (64 if h >= 2 else 0)
                for kb in range(nkb):
                    nc.tensor.matmul(po[:], vaug[kb][:, ws:ws + 128],
                                     es[kb][:],
                                     start=(kb == 0), stop=(kb == nkb - 1))
                if h % 2 == 0:
                    nrows, crows = po[0:64, :], po[64:128, :]
                else:
                    nrows, crows = po[64:128, :], po[0:64, :]
                cs = csp.tile([64, HCW], F32, name="cs")
                nc.scalar.activation(cs[:], crows, AF.Identity,
                                     bias=bias_sb[0:64, COL_EPS:COL_EPS + 1])
                rc = csp.tile([64, HCW], F32, name="rc")
                nc.vector.reciprocal_approx_fast(out=rc[:], in_=cs[:])
                cs2 = csp.tile([64, HCW], F32, name="cs2")
                nc.vector.tensor_tensor(cs2[:], nrows, rc[:], op=OP.mult)
                nc.scalar.activation(
                    hd[o][ro:ro + 64, :], cs2[:], AF.Identity,
                    bias=bias_sb[ro:ro + 64, bvc + o:bvc + o + 1])
            if causal and ch == 0:
                # position 0 attends to nothing -> exactly zero output
                for o in range(2):
                    nc.vector.tensor_scalar_mul(hd[o][:, 0:1],
                                                hd[o][:, 0:1], 0.0)
            return hd

        def partial_ar(src, wsel, mode, nsrc):
            """src: list of tiles [128, HCW] (contraction tiles);
            wsel cols: i*1024 + dout*128.  AllReduce [D, HCW] bf16."""
            arin = dram.tile([D, HCW], BF16, name="arin")
            arout = dram.tile([D, HCW], BF16, name="arout")
            ocw = aop.tile([128, DT * HCW], BF16, name="ocw")
            for half in range(2):
                for dout in range(half * 4, half * 4 + 4):
                    pwf = ps.tile([128, TL], F32, name="pw", tag="mm")
                    pw = pwf[:, 0:HCW]
                    for i in range(nsrc):
                        c0 = i * D + dout * 128
                        nc.tensor.matmul(pw[:], wsel[:, c0:c0 + 128],
                                         src[i][:],
                                         start=(i == 0), stop=(i == nsrc - 1))
                    osl = ocw[:, dout * HCW:(dout + 1) * HCW]
                    if dout % 2 == 0:
                        nc.scalar.activation(osl, pw[:], AF.Copy)
                    else:
                        nc.vector.tensor_copy(osl, pw[:])
                h0 = half * 4
                nc.sync.dma_start(
                    arin[h0 * 128:(h0 + 4) * 128, :].rearrange(
                        "(dt p) t -> p dt t", p=128),
                    ocw[:, h0 * HCW:(h0 + 4) * HCW].rearrange(
                        "p (dt t) -> p dt t", t=HCW))
            import os as _os
            nlim = int(_os.environ.get("K2_CC_N", "999"))
            _CCCNT[0] += 1
            if _os.environ.get("K2_SKIP_CC") == "1" or _CCCNT[0] > nlim:
                return arin
            rg = [[0, 1, 2, 3, 4, 5, 6, 7]] \
                if _os.environ.get("K2_RG8") == "1" else RG
            nc.gpsimd.collective_compute("AllReduce", OP.add,
                                         replica_groups=rg,
                                         ins=[arin[:]], outs=[arout[:]])
            return arout

        def boundary(ch, arout, bnd):
            art = arp.tile([128, DT * HCW], BF16, name="art")
            for half in range(2):
                h0 = half * 4
                nc.sync.dma_start(
                    art[:, h0 * HCW:(h0 + 4) * HCW].rearrange(
                        "p (dt t) -> p dt t", t=HCW),
                    arout[h0 * 128:(h0 + 4) * 128, :].rearrange(
                        "(dt p) t -> p dt t", p=128))
            for dt in range(DT):
                # bf16 shadow on the critical path (DVE), fp32 residual
                # update off-path on the otherwise-idle GpSimd engine
                nc.vector.scalar_tensor_tensor(
                    zb[dt][:, hc(ch)], z[dt][:, hc(ch)],
                    bcol(COL_SIG + bnd * 8 + dt),
                    art[:, dt * HCW:(dt + 1) * HCW], OP.mult, OP.add)
                nc.vector.scalar_tensor_tensor(
                    z[dt][:, hc(ch)], z[dt][:, hc(ch)],
                    bcol(COL_SIG + bnd * 8 + dt),
                    art[:, dt * HCW:(dt + 1) * HCW], OP.mult, OP.add)

        def proj2(srcs, wsl, bc0, nm, pool):
            """2 out-tiles [128, HCW] = w.T @ src (+bias)."""
            outs = []
            for o in range(2):
                ppf = ps.tile([128, TL], F32, name="pp", tag="mm")
                pp = ppf[:, 0:HCW]
                for dt in range(DT):
                    nc.tensor.matmul(pp[:], wsl(dt, o), srcs(dt),
                                     start=(dt == 0), stop=(dt == DT - 1))
                pt = pool.tile([128, HCW], BF16, name=f"{nm}{o}")
                nc.scalar.activation(pt[:], pp[:], AF.Identity,
                                     bias=bcol(bc0 + o))
                outs.append(pt)
            return outs

        for l in range(L):
            aw_s = wap.tile([128, DT * 3 * EL], BF16, name="aw")
            nc.sync.dma_start(aw_s[:], attw_s[l])
            wo_s_t = wop.tile([128, 2 * D], BF16, name="wot")
            nc.sync.dma_start(wo_s_t[:], wo_s[l])
            aq_c = waqc.tile([128, DT * EL], BF16, name="aqc")
            nc.sync.dma_start(aq_c[:], attq_c[l])
            akv_c = wakv.tile([128, DT * 2 * EL], BF16, name="akvc")
            nc.sync.dma_start(akv_c[:], attkv_c[l])

            # self attention per half-chunk
            k_t = [kvp.tile([128, TL], BF16, name=f"kt{o}") for o in range(2)]
            vaug = [None] * 4
            ars = []
            for ch in range(CH):
                q_t = []
                for o in range(2):
                    ppf = ps.tile([128, TL], F32, name="pq", tag="mm")
                    pp = ppf[:, 0:HCW]
                    for dt in range(DT):
                        c0 = dt * 768 + 0 * 256 + o * 128
                        nc.tensor.matmul(pp[:], aw_s[:, c0:c0 + 128],
                                         zb[dt][:, hc(ch)],
                                         start=(dt == 0), stop=(dt == DT - 1))
                    qt = qp.tile([128, HCW], BF16, name=f"qt{o}")
                    nc.scalar.activation(qt[:], pp[:], AF.Identity,
                                         bias=bcol(COL_QKV + l * 12 + o))
                    q_t.append(qt)
                for o in range(2):
                    ppf = ps.tile([128, TL], F32, name="pkv", tag="mm")
                    pp = ppf[:, 0:HCW]
                    for dt in range(DT):
                        c0 = dt * 768 + 256 + o * 128
                        nc.tensor.matmul(pp[:], aw_s[:, c0:c0 + 128],
                                         zb[dt][:, hc(ch)],
                                         start=(dt == 0),
                                         stop=(dt == DT - 1))
                    nc.scalar.activation(
                        k_t[o][:, hc(ch)], pp[:], AF.Identity,
                        bias=bcol(COL_QKV + l * 12 + 2 + o))
                for kb in range(2 * ch, 2 * ch + 2):
                    vaug[kb] = build_vaug(
                        kb, vap,
                        lambda dt, kb=kb: zb[dt][:, kb * 128:(kb + 1) * 128],
                        lambda dt, o: aw_s[:, dt * 768 + 512 + o * 128:
                                           dt * 768 + 512 + o * 128 + 128])
                hd = attn_core(q_t, k_t, vaug, ch, True,
                               COL_QKV + l * 12 + 4)
                ars.append(partial_ar(hd, wo_s_t, "wo", 2))

            # hoisted cross k/v (full 512 tokens, encoder-sourced)
            ck_t = [ckvp.tile([128, TL], BF16, name=f"ck{o}")
                    for o in range(2)]
            for o in range(2):
                pp = ps.tile([128, TL], F32, name="pck", tag="mm")
                for dt in range(DT):
                    c0 = dt * 512 + o * 128
                    nc.tensor.matmul(pp[:], akv_c[:, c0:c0 + 128],
                                     encs[dt][:],
                                     start=(dt == 0), stop=(dt == DT - 1))
                nc.scalar.activation(
                    ck_t[o][:], pp[:], AF.Identity,
                    bias=bcol(COL_QKV + l * 12 + 8 + o))
            cvaug = [build_vaug(
                kb, cvap,
                lambda dt, kb=kb: encs[dt][:, kb * 128:(kb + 1) * 128],
                lambda dt, o: akv_c[:, dt * 512 + 256 + o * 128:
                                    dt * 512 + 256 + o * 128 + 128])
                for kb in range(4)]

            wo_c_t = wop.tile([128, 2 * D], BF16, name="woc")
            nc.sync.dma_start(wo_c_t[:], wo_c[l])
            w1t = w1pool.tile([128, DT * FFL], BF16, name="w1t")
            nc.sync.dma_start(w1t[:], w1p[l])
            w2t = w2pool.tile([128, HT * D], BF16, name="w2t")
            nc.sync.dma_start(w2t[:], w2p[l])
            arc = []
            for ch in range(CH):
                boundary(ch, ars[ch], 3 * l)
                zsrc = lambda dt, ch=ch: zb[dt][:, hc(ch)]
                q_c = proj2(zsrc,
                            lambda dt, o: aq_c[:, dt * 256 + o * 128:
                                               dt * 256 + o * 128 + 128],
                            COL_QKV + l * 12 + 6, "qc", qp)
                hd = attn_core(q_c, ck_t, cvaug, ch, False,
                               COL_QKV + l * 12 + 10)
                arc.append(partial_ar(hd, wo_c_t, "wo", 2))

            # FFN per half-chunk
            arf = []
            for ch in range(CH):
                boundary(ch, arc[ch], 3 * l + 1)
                hts = []
                for ht in range(HT):
                    ppf = ps.tile([128, TL], F32, name="pf", tag="mm")
                    pp = ppf[:, 0:HCW]
                    for dt in range(DT):
                        c0 = dt * FFL + ht * 128
                        nc.tensor.matmul(pp[:], w1t[:, c0:c0 + 128],
                                         zb[dt][:, hc(ch)],
                                         start=(dt == 0), stop=(dt == DT - 1))
                    htile = hfp.tile([128, HCW], BF16, name=f"hf{ht}")
                    nc.scalar.activation(htile[:], pp[:], AF.Relu,
                                         bias=bcol(COL_B1 + l * 8 + ht))
                    hts.append(htile)
                arf.append(partial_ar(hts, w2t, "ffn2", HT))
            for ch in range(CH):
                boundary(ch, arf[ch], 3 * l + 2)

        # vocab projection: first NCHUNKED slices run on half-chunk 0
        # only (early start under the final ARs), then their half 1, then
        # the remainder full-width N=512
        NCHUNKED = 12

        def vocab_slice(vs, cols, wt):
            w = cols.stop - cols.start
            ppf = ps.tile([128, TL], F32, name="pv", tag="mm")
            pp = ppf[:, 0:w]
            for dt in range(DT):
                nc.tensor.matmul(pp[:], wt[:, dt * 128:(dt + 1) * 128],
                                 zb[dt][:, cols],
                                 start=(dt == 0), stop=(dt == DT - 1))
            osb = osp.tile([128, TL], F32, name="osb")
            osl = osb[:, 0:w]
            if vs % 2 == 0:
                nc.scalar.activation(osl, pp[:], AF.Identity,
                                     bias=bcol(COL_BOUT + vs))
            else:
                nc.vector.tensor_scalar_add(osl, pp[:], bcol(COL_BOUT + vs))
            nc.sync.dma_start(logt[vs * 128:(vs + 1) * 128, cols], osl)

        for ph, (v0, v1, cols) in enumerate([
                (0, NCHUNKED, hc(0)), (0, NCHUNKED, hc(1)),
                (NCHUNKED, VS, slice(0, TL))]):
            for vs in range(v0, v1):
                wt = wvp.tile([128, D], BF16, name="wv")
                nc.sync.dma_start(wt[:], woutp[:, vs * D:(vs + 1) * D])
                vocab_slice(vs, cols, wt)
    nc.compile()
    return nc


def _host_prepare(inputs):
    f = lambda a: np.asarray(a, dtype=np.float64)
    tobf = lambda a: a.astype(ml_dtypes.bfloat16)
    seq = np.asarray(inputs["sequence"])
    emb = np.asarray(inputs["emb"], dtype=np.float32)
    pes = np.asarray(inputs["pes"], dtype=np.float32)
    enc = np.asarray(inputs["encoder_out"], dtype=np.float32)

    x0 = emb[seq] + pes[None, :, :]                   # [B, S, D]
    xts = [np.ascontiguousarray(x0[b].T.astype(np.float32))
           for b in range(B)]                         # [D, S] per batch
    encts = [np.ascontiguousarray(tobf(enc[b].T)) for b in range(B)]

    mask = (np.arange(128)[:, None] < np.arange(128)[None, :])
    maskd = np.ascontiguousarray(tobf(mask.astype(np.float32)))

    bf = ml_dtypes.bfloat16
    attw_s = np.zeros((TPW, L, 128, DT * 3 * EL), bf)
    attq_c = np.zeros((TPW, L, 128, DT * EL), bf)
    attkv_c = np.zeros((TPW, L, 128, DT * 2 * EL), bf)
    wo_s_p = np.zeros((TPW, L, 128, 2 * D), bf)
    wo_c_p = np.zeros((TPW, L, 128, 2 * D), bf)
    w1pp = np.zeros((TPW, L, 128, DT * FFL), bf)
    w2pp = np.zeros((TPW, L, 128, HT * D), bf)
    woutpp = np.zeros((TPW, 128, VS * D), bf)
    biaspp = np.zeros((TPW, 128, NBCOL), np.float32)

    def pack_kxm(w, ncols):
        kt = w.shape[0] // 128
        return w.reshape(kt, 128, ncols).transpose(1, 0, 2).reshape(
            128, kt * ncols)

    sig = np.ones(D)
    gam = np.zeros(D)
    for l in range(L):
        for which, (wq, bq, wk, bk, wv, bv, wo, bo, g, be, m, v) in enumerate([
            (inputs["wq_s"][l], inputs["bq_s"][l], inputs["wk_s"][l],
             inputs["bk_s"][l], inputs["wv_s"][l], inputs["bv_s"][l],
             inputs["wo_s"][l], inputs["bo_s"][l], inputs["g1"][l],
             inputs["be1"][l], inputs["m1"][l], inputs["v1"][l]),
            (inputs["wq_c"][l], inputs["bq_c"][l], inputs["wk_c"][l],
             inputs["bk_c"][l], inputs["wv_c"][l], inputs["bv_c"][l],
             inputs["wo_c"][l], inputs["bo_c"][l], inputs["g2"][l],
             inputs["be2"][l], inputs["m2"][l], inputs["v2"][l]),
        ]):
            wq, wk, wv = f(wq), f(wk), f(wv)          # [H, D, DH]
            bq, bk, bv = f(bq), f(bk), f(bv)          # [H, DH]
            wo, bo = f(wo), f(bo)
            for r in range(TPW):
                h0 = r * HL
                wql = wq[h0:h0 + HL].transpose(1, 0, 2).reshape(D, EL)
                wkl = wk[h0:h0 + HL].transpose(1, 0, 2).reshape(D, EL)
                wvl = wv[h0:h0 + HL].transpose(1, 0, 2).reshape(D, EL)
                bql = bq[h0:h0 + HL].reshape(EL)
                bkl = bk[h0:h0 + HL].reshape(EL)
                bvl = bv[h0:h0 + HL].reshape(EL)
                wq_eff = (sig[:, None] * wql) / 8.0
                bq_eff = (gam @ wql + bql) / 8.0
                if which == 0:
                    wk_eff = sig[:, None] * wkl
                    bk_eff = gam @ wkl + bkl
                    wv_eff = sig[:, None] * wvl
                    bv_eff = gam @ wvl + bvl
                    wcat = np.concatenate([wq_eff, wk_eff, wv_eff], axis=1)
                    attw_s[r, l] = tobf(
                        pack_kxm(wcat, 3 * EL).astype(np.float32))
                    cb = COL_QKV + l * 12
                    for o in range(2):
                        biaspp[r, :, cb + 0 + o] = \
                            bq_eff[o * 128:(o + 1) * 128]
                        biaspp[r, :, cb + 2 + o] = \
                            bk_eff[o * 128:(o + 1) * 128]
                        biaspp[r, :, cb + 4 + o] = \
                            bv_eff[o * 128:(o + 1) * 128]
                else:
                    attq_c[r, l] = tobf(
                        pack_kxm(wq_eff, EL).astype(np.float32))
                    kvcat = np.concatenate([wkl, wvl], axis=1)
                    attkv_c[r, l] = tobf(
                        pack_kxm(kvcat, 2 * EL).astype(np.float32))
                    cb = COL_QKV + l * 12
                    for o in range(2):
                        biaspp[r, :, cb + 6 + o] = \
                            bq_eff[o * 128:(o + 1) * 128]
                        biaspp[r, :, cb + 8 + o] = bkl[o * 128:(o + 1) * 128]
                        biaspp[r, :, cb + 10 + o] = bvl[o * 128:(o + 1) * 128]
                wo_loc = wo[r * EL:(r + 1) * EL, :]       # [256, 1024]
                wo_pk = wo_loc.reshape(2, 128, D).transpose(1, 0, 2).reshape(
                    128, 2 * D)
                (wo_s_p if which == 0 else wo_c_p)[r, l] = tobf(
                    wo_pk.astype(np.float32))
            bnd = 3 * l + which
            for r in range(TPW):
                for dt in range(DT):
                    biaspp[r, :, COL_SIG + bnd * 8 + dt] = \
                        sig[dt * 128:(dt + 1) * 128].astype(np.float32)
            beta = gam + bo
            s = f(g) / np.sqrt(f(v) + EPS)
            cshift = f(be) - f(m) * s
            sig = s
            gam = s * beta + cshift

        w1, b1 = f(inputs["w1"][l]), f(inputs["b1"][l])
        w2, b2 = f(inputs["w2"][l]), f(inputs["b2"][l])
        for r in range(TPW):
            cols = slice(r * FFL, (r + 1) * FFL)
            w1_eff = sig[:, None] * w1[:, cols]
            b1_eff = gam @ w1[:, cols] + b1[cols]
            w1pp[r, l] = tobf(pack_kxm(w1_eff, FFL).astype(np.float32))
            w2pp[r, l] = tobf(pack_kxm(w2[cols, :], D).astype(np.float32))
            for ht in range(HT):
                biaspp[r, :, COL_B1 + l * 8 + ht] = \
                    b1_eff[ht * 128:(ht + 1) * 128].astype(np.float32)
        bnd = 3 * l + 2
        for r in range(TPW):
            for dt in range(DT):
                biaspp[r, :, COL_SIG + bnd * 8 + dt] = \
                    sig[dt * 128:(dt + 1) * 128].astype(np.float32)
        beta = gam + b2
        s = f(inputs["g3"][l]) / np.sqrt(f(inputs["v3"][l]) + EPS)
        cshift = f(inputs["be3"][l]) - f(inputs["m3"][l]) * s
        sig = s
        gam = s * beta + cshift

    wout, bout = f(inputs["w_out"]), f(inputs["b_out"])
    for r in range(TPW):
        wsl = np.zeros((D, VPAD))
        bsl = np.zeros(VPAD)
        cols = slice(r * VL, (r + 1) * VL)
        wsl[:, :VL] = wout[:, cols]
        bsl[:VL] = bout[cols]
        wout_eff = sig[:, None] * wsl
        bout_eff = gam @ wsl + bsl
        woutpp[r] = tobf(wout_eff.reshape(DT, 128, VS, 128).transpose(
            1, 2, 0, 3).reshape(128, VS * D).astype(np.float32))
        for vs in range(VS):
            biaspp[r, :, COL_BOUT + vs] = \
                bout_eff[vs * 128:(vs + 1) * 128].astype(np.float32)

    biaspp[:, :, COL_EPS] = 1e-30
    in_maps = []
    for c in range(NC):
        g, r = c // TPW, c % TPW
        in_maps.append({
            "xt": xts[g], "enct": encts[g],
            "attw_s": attw_s[r], "attq_c": attq_c[r], "attkv_c": attkv_c[r],
            "wo_s": wo_s_p[r], "wo_c": wo_c_p[r],
            "w1p": w1pp[r], "w2p": w2pp[r], "woutp": woutpp[r],
            "biasp": biaspp[r], "maskd": maskd,
            "identd": tobf(np.eye(128, dtype=np.float32)),
            "onesd": np.ones((128, 64), dtype=ml_dtypes.bfloat16),
        })
    return in_maps


_NC_CACHE = {}


def _get_program():
    if "nc" not in _NC_CACHE:
        _NC_CACHE["nc"] = _build_program()
    return _NC_CACHE["nc"]


def run(inputs, trace=False):
    nc = _get_program()
    in_maps = _host_prepare(inputs)
    res = bass_utils.run_bass_kernel_spmd(nc, in_maps, list(range(NC)),
                                          trace=trace)
    out = np.empty((B, S, V), np.float32)
    for c in range(NC):
        g, r = c // TPW, c % TPW
        out[g, :, r * VL:(r + 1) * VL] = res.results[c]["logt"][:VL, :].T
    return out, res


def kernel(**inputs):
    out, _ = run(inputs)
    return out


# revision 5
# speedup vs baseline: 1.0212x; 1.0042x over previous
"""Trainium2 Bass kernel, v2: TP4 x DP2 for the 4-layer decoder.

  - Cores 0-3 handle batch 0, cores 4-7 batch 1 (disjoint 4-rank
    replica groups -> the two groups' collectives run concurrently).
  - Within a group: tensor-parallel over 4 heads/core (EL=256),
    FFN 1024 hidden/core, vocab 8000 cols/core.
  - Each batch's 512 tokens split into two 256-token half-chunks;
    boundary AllReduces are per-half (512KB bf16, 4-rank Mesh) so the
    PE computes one half while the other half's AR is in flight.
  - All matmuls bf16 (residual z kept fp32 on SBUF with a bf16 shadow
    zb used as matmul rhs).  BN + biases folded host-side as before.
"""

import sys
import numpy as np

if "/opt/trn_rl_repo" not in sys.path:
    sys.path.insert(0, "/opt/trn_rl_repo")

import ml_dtypes
import concourse.bass as bass
import concourse.mybir as mybir
import concourse.tile as tile
from concourse import bacc
from concourse import bass_utils

V, D, H, L, B, S, SE = 32000, 1024, 16, 4, 2, 512, 512
DH = D // H            # 64
EPS = 1e-3
NC = 8
TPW = 4                # tensor-parallel width (per group)
HL = H // TPW          # 4 heads per core
EL = HL * DH           # 256 local head dims
FF = 4 * D
FFL = FF // TPW        # 1024 ffn hidden per core
VL = V // TPW          # 8000
VPAD = 8192
VS = VPAD // 128       # 64 vocab slices
TL = S                 # 512 tokens per group (one batch)
CH = 2                 # half-chunks
HCW = TL // CH         # 256
DT = D // 128          # 8
HT = FFL // 128        # 8
NBND = 3 * L           # 12

F32 = mybir.dt.float32
BF16 = mybir.dt.bfloat16
AF = mybir.ActivationFunctionType
OP = mybir.AluOpType

RG = [[0, 1, 2, 3], [4, 5, 6, 7]]
_CCCNT = [0]

# bias-tile column layout
COL_QKV = 0                      # L*12: l*12 + proj(6)*2 + o
COL_B1 = COL_QKV + 12 * L        # L*8: l*8 + ht
COL_SIG = COL_B1 + 8 * L         # 12*8: bnd*8 + dt
COL_BOUT = COL_SIG + 8 * NBND    # 64
COL_EPS = COL_BOUT + VS
NBCOL = COL_EPS + 1


def _build_program():
    nc = bacc.Bacc("TRN2", target_bir_lowering=False, debug=False,
                   num_devices=NC)
    dd = lambda name, shape, dtype=BF16, kind="ExternalInput": \
        nc.dram_tensor(name, shape, dtype, kind=kind).ap()

    xt = dd("xt", [D, TL], F32)
    enct = dd("enct", [D, TL])
    attw_s = dd("attw_s", [L, 128, DT * 3 * EL])    # col = dt*768+p*256+o*128
    attq_c = dd("attq_c", [L, 128, DT * EL])        # col = dt*256 + o*128
    attkv_c = dd("attkv_c", [L, 128, DT * 2 * EL])  # col = dt*512+{k0,v256}+o*128
    wo_s = dd("wo_s", [L, 128, 2 * D])              # col = kt*1024 + dout
    wo_c = dd("wo_c", [L, 128, 2 * D])
    w1p = dd("w1p", [L, 128, DT * FFL])             # col = dt*1024 + f
    w2p = dd("w2p", [L, 128, HT * D])               # col = ht*1024 + dout
    woutp = dd("woutp", [128, VS * D])              # col = vs*1024 + dt*128 + j
    biasp = dd("biasp", [128, NBCOL], F32)
    maskd = dd("maskd", [128, 128])                 # strictly-lower 0/1
    identd = dd("identd", [128, 128])
    onesd = dd("onesd", [128, 64])
    logt = dd("logt", [VPAD, TL], F32, kind="ExternalOutput")

    from contextlib import ExitStack
    with tile.TileContext(nc) as tc, ExitStack() as _es:
        P = lambda **kw: _es.enter_context(tc.tile_pool(**kw))
        cst = P(name="cst", bufs=1)
        zp = P(name="zp", bufs=1)
        zbp = P(name="zbp", bufs=1)
        encp = P(name="encp", bufs=1)
        qp = P(name="qp", bufs=2)
        kvp = P(name="kvp", bufs=2)
        ckvp = P(name="ckvp", bufs=2)
        vap = P(name="vap", bufs=2)
        cvap = P(name="cvap", bufs=2)
        esp = P(name="esp", bufs=5)
        hdp = P(name="hdp", bufs=2)
        csp = P(name="csp", bufs=2)
        hfp = P(name="hfp", bufs=2)
        arp = P(name="arp", bufs=2)
        aop = P(name="aop", bufs=2)
        wap = P(name="wap", bufs=2)
        waqc = P(name="waqc", bufs=1)
        wakv = P(name="wakv", bufs=1)
        wop = P(name="wop", bufs=1)
        w1pool = P(name="w1pool", bufs=1)
        w2pool = P(name="w2pool", bufs=1)
        wvp = P(name="wvp", bufs=16)
        osp = P(name="osp", bufs=3)
        ps = P(name="ps", bufs=8, space="PSUM")
        dram = P(name="dram", bufs=4, space="DRAM")

        bias_sb = cst.tile([128, NBCOL], F32)
        nc.sync.dma_start(bias_sb[:], biasp[:])
        mask_sb = cst.tile([128, 128], BF16)
        nc.sync.dma_start(mask_sb[:], maskd[:])
        ident = cst.tile([128, 128], BF16)
        nc.sync.dma_start(ident[:], identd[:])
        ones_sb = cst.tile([128, 64], BF16)
        nc.sync.dma_start(ones_sb[:], onesd[:])
        zeros_sb = cst.tile([128, 256], BF16)
        nc.vector.tensor_scalar_mul(zeros_sb[:, 0:64], ones_sb[:], 0.0)
        nc.vector.tensor_copy(zeros_sb[:, 64:128], zeros_sb[:, 0:64])
        nc.vector.tensor_copy(zeros_sb[:, 128:256], zeros_sb[:, 0:128])

        z = [None] * DT
        zb = [None] * DT
        for dt in range(DT):
            zt = zp.tile([128, TL], F32, name=f"z{dt}")
            nc.sync.dma_start(zt[:], xt[dt * 128:(dt + 1) * 128, :])
            z[dt] = zt
            zbt = zbp.tile([128, TL], BF16, name=f"zb{dt}")
            if dt % 2 == 0:
                nc.scalar.activation(zbt[:], zt[:], AF.Copy)
            else:
                nc.vector.tensor_copy(zbt[:], zt[:])
            zb[dt] = zbt

        # encoder activations loaded after z (needed only at cross-attn)
        encs = []
        for dt in range(DT):
            et = encp.tile([128, TL], BF16, name=f"enc{dt}")
            nc.sync.dma_start(et[:], enct[dt * 128:(dt + 1) * 128, :])
            encs.append(et)

        def bcol(c):
            return bias_sb[:, c:c + 1]

        def hc(ch):
            return slice(ch * HCW, (ch + 1) * HCW)

        def build_vaug(kb, pool, vsrc, vwsl):
            """Token-major vaug [128, 384] built straight from the v
            projection: partitions = tokens of key-block kb, cols =
            [h0|ones|h1|h2|ones|h3].  vsrc(dt) is the [128, 128] lhsT
            slice of the activation source for this kb; vwsl(dt, o) the
            [128, 128] v-weight slice for feature half o."""
            va = pool.tile([128, 384], BF16, name=f"va{kb}")
            for o in range(2):
                ppf = ps.tile([128, TL], F32, name="pvt", tag="mm")
                pp = ppf[:, 0:128]
                for dt in range(DT):
                    nc.tensor.matmul(pp[:], vsrc(dt), vwsl(dt, o),
                                     start=(dt == 0), stop=(dt == DT - 1))
                base = 192 * o
                nc.scalar.activation(va[:, base:base + 64], pp[:, 0:64],
                                     AF.Identity)
                dsth1 = va[:, 128:192] if o == 0 else va[:, 320:384]
                nc.scalar.activation(dsth1, pp[:, 64:128], AF.Identity)
            nc.vector.tensor_copy(va[:, 64:128], ones_sb[:])
            nc.vector.tensor_copy(va[:, 256:320], ones_sb[:])
            return va

        def attn_core(qt, kt, vaug, ch, causal, bvc):
            """qt: 2x[128,HCW]; kt: 2x[128,TL]; vaug: list of [128,384].
            bvc: bias column base for the deferred v bias (+o).
            Returns hd: 2 tiles [128, HCW] bf16."""
            nkb = 2 * (ch + 1) if causal else 4
            hd = [hdp.tile([128, HCW], BF16, name=f"hd{o}") for o in range(2)]
            for h in range(HL):
                o, ro = h // 2, (h % 2) * 64
                es = []
                for kb in range(nkb):
                    q0 = max(0, kb * 128 - ch * HCW) if causal else 0
                    ppf = ps.tile([128, TL], F32, name="psc", tag="mm")
                    pp = ppf[:, 0:HCW]
                    nc.tensor.matmul(pp[:],
                                     kt[o][ro:ro + 64,
                                           kb * 128:(kb + 1) * 128],
                                     qt[o][ro:ro + 64, :],
                                     start=True, stop=True)
                    et = esp.tile([128, HCW], BF16, name="es")
                    nc.scalar.activation(et[:, q0:HCW], pp[:, q0:HCW], AF.Exp)
                    if causal and kb * 128 >= ch * HCW:
                        if q0 > 0:
                            nc.vector.tensor_copy(et[:, 0:q0],
                                                  zeros_sb[:, 0:q0])
                        nc.vector.tensor_tensor(et[:, q0:q0 + 128],
                                                et[:, q0:q0 + 128],
                                                mask_sb[:], op=OP.mult)
                    es.append(et)
                pof = ps.tile([128, TL], F32, name="po", tag="mm")
                po = pof[:, 0:HCW]
                # stationary window: [val|ones] for even h, [ones|val] odd
                ws = 64 * h + (64 if h >= 2 else 0)
                for kb in range(nkb):
                    nc.tensor.matmul(po[:], vaug[kb][:, ws:ws + 128],
                                     es[kb][:],
                                     start=(kb == 0), stop=(kb == nkb - 1))
                if h % 2 == 0:
                    nrows, crows = po[0:64, :], po[64:128, :]
                else:
                    nrows, crows = po[64:128, :], po[0:64, :]
                cs = csp.tile([64, HCW], F32, name="cs")
                nc.scalar.activation(cs[:], crows, AF.Identity,
                                     bias=bias_sb[0:64, COL_EPS:COL_EPS + 1])
                rc = csp.tile([64, HCW], F32, name="rc")
                nc.vector.reciprocal_approx_fast(out=rc[:], in_=cs[:])
                cs2 = csp.tile([64, HCW], F32, name="cs2")
                nc.vector.tensor_tensor(cs2[:], nrows, rc[:], op=OP.mult)
                nc.scalar.activation(
                    hd[o][ro:ro + 64, :], cs2[:], AF.Identity,
                    bias=bias_sb[ro:ro + 64, bvc + o:bvc + o + 1])
            if causal and ch == 0:
                # position 0 attends to nothing -> exactly zero output
                for o in range(2):
                    nc.vector.tensor_scalar_mul(hd[o][:, 0:1],
                                                hd[o][:, 0:1], 0.0)
            return hd

        def partial_ar(src, wsel, mode, nsrc):
            """src: list of tiles [128, HCW] (contraction tiles);
            wsel cols: i*1024 + dout*128.  AllReduce [D, HCW] bf16."""
            arin = dram.tile([D, HCW], BF16, name="arin")
            arout = dram.tile([D, HCW], BF16, name="arout")
            ocw = aop.tile([128, DT * HCW], BF16, name="ocw")
            for half in range(2):
                for dout in range(half * 4, half * 4 + 4):
                    pwf = ps.tile([128, TL], F32, name="pw", tag="mm")
                    pw = pwf[:, 0:HCW]
                    for i in range(nsrc):
                        c0 = i * D + dout * 128
                        nc.tensor.matmul(pw[:], wsel[:, c0:c0 + 128],
                                         src[i][:],
                                         start=(i == 0), stop=(i == nsrc - 1))
                    osl = ocw[:, dout * HCW:(dout + 1) * HCW]
                    if dout % 2 == 0:
                        nc.scalar.activation(osl, pw[:], AF.Copy)
                    else:
                        nc.vector.tensor_copy(osl, pw[:])
                h0 = half * 4
                nc.sync.dma_start(
                    arin[h0 * 128:(h0 + 4) * 128, :].rearrange(
                        "(dt p) t -> p dt t", p=128),
                    ocw[:, h0 * HCW:(h0 + 4) * HCW].rearrange(
                        "p (dt t) -> p dt t", t=HCW))
            import os as _os
            nlim = int(_os.environ.get("K2_CC_N", "999"))
            _CCCNT[0] += 1
            if _os.environ.get("K2_SKIP_CC") == "1" or _CCCNT[0] > nlim:
                return arin
            rg = [[0, 1, 2, 3, 4, 5, 6, 7]] \
                if _os.environ.get("K2_RG8") == "1" else RG
            nc.gpsimd.collective_compute("AllReduce", OP.add,
                                         replica_groups=rg,
                                         ins=[arin[:]], outs=[arout[:]])
            return arout

        def boundary(ch, arout, bnd):
            art = arp.tile([128, DT * HCW], BF16, name="art")
            for half in range(2):
                h0 = half * 4
                nc.sync.dma_start(
                    art[:, h0 * HCW:(h0 + 4) * HCW].rearrange(
                        "p (dt t) -> p dt t", t=HCW),
                    arout[h0 * 128:(h0 + 4) * 128, :].rearrange(
                        "(dt p) t -> p dt t", p=128))
            for dt in range(DT):
                # bf16 shadow on the critical path (DVE), fp32 residual
                # update off-path on the otherwise-idle GpSimd engine
                nc.vector.scalar_tensor_tensor(
                    zb[dt][:, hc(ch)], z[dt][:, hc(ch)],
                    bcol(COL_SIG + bnd * 8 + dt),
                    art[:, dt * HCW:(dt + 1) * HCW], OP.mult, OP.add)
                nc.vector.scalar_tensor_tensor(
                    z[dt][:, hc(ch)], z[dt][:, hc(ch)],
                    bcol(COL_SIG + bnd * 8 + dt),
                    art[:, dt * HCW:(dt + 1) * HCW], OP.mult, OP.add)

        def proj2(srcs, wsl, bc0, nm, pool):
            """2 out-tiles [128, HCW] = w.T @ src (+bias)."""
            outs = []
            for o in range(2):
                ppf = ps.tile([128, TL], F32, name="pp", tag="mm")
                pp = ppf[:, 0:HCW]
                for dt in range(DT):
                    nc.tensor.matmul(pp[:], wsl(dt, o), srcs(dt),
                                     start=(dt == 0), stop=(dt == DT - 1))
                pt = pool.tile([128, HCW], BF16, name=f"{nm}{o}")
                nc.scalar.activation(pt[:], pp[:], AF.Identity,
                                     bias=bcol(bc0 + o))
                outs.append(pt)
            return outs

        for l in range(L):
            aw_s = wap.tile([128, DT * 3 * EL], BF16, name="aw")
            nc.sync.dma_start(aw_s[:], attw_s[l])
            wo_s_t = wop.tile([128, 2 * D], BF16, name="wot")
            nc.sync.dma_start(wo_s_t[:], wo_s[l])
            aq_c = waqc.tile([128, DT * EL], BF16, name="aqc")
            nc.sync.dma_start(aq_c[:], attq_c[l])
            akv_c = wakv.tile([128, DT * 2 * EL], BF16, name="akvc")
            nc.sync.dma_start(akv_c[:], attkv_c[l])

            # self attention per half-chunk
            k_t = [kvp.tile([128, TL], BF16, name=f"kt{o}") for o in range(2)]
            vaug = [None] * 4
            ars = []
            for ch in range(CH):
                q_t = []
                for o in range(2):
                    ppf = ps.tile([128, TL], F32, name="pq", tag="mm")
                    pp = ppf[:, 0:HCW]
                    for dt in range(DT):
                        c0 = dt * 768 + 0 * 256 + o * 128
                        nc.tensor.matmul(pp[:], aw_s[:, c0:c0 + 128],
                                         zb[dt][:, hc(ch)],
                                         start=(dt == 0), stop=(dt == DT - 1))
                    qt = qp.tile([128, HCW], BF16, name=f"qt{o}")
                    nc.scalar.activation(qt[:], pp[:], AF.Identity,
                                         bias=bcol(COL_QKV + l * 12 + o))
                    q_t.append(qt)
                for o in range(2):
                    ppf = ps.tile([128, TL], F32, name="pkv", tag="mm")
                    pp = ppf[:, 0:HCW]
                    for dt in range(DT):
                        c0 = dt * 768 + 256 + o * 128
                        nc.tensor.matmul(pp[:], aw_s[:, c0:c0 + 128],
                                         zb[dt][:, hc(ch)],
                                         start=(dt == 0),
                                         stop=(dt == DT - 1))
                    nc.scalar.activation(
                        k_t[o][:, hc(ch)], pp[:], AF.Identity,
                        bias=bcol(COL_QKV + l * 12 + 2 + o))
                for kb in range(2 * ch, 2 * ch + 2):
                    vaug[kb] = build_vaug(
                        kb, vap,
                        lambda dt, kb=kb: zb[dt][:, kb * 128:(kb + 1) * 128],
                        lambda dt, o: aw_s[:, dt * 768 + 512 + o * 128:
                                           dt * 768 + 512 + o * 128 + 128])
                hd = attn_core(q_t, k_t, vaug, ch, True,
                               COL_QKV + l * 12 + 4)
                ars.append(partial_ar(hd, wo_s_t, "wo", 2))

            # hoisted cross k/v (full 512 tokens, encoder-sourced)
            ck_t = [ckvp.tile([128, TL], BF16, name=f"ck{o}")
                    for o in range(2)]
            for o in range(2):
                pp = ps.tile([128, TL], F32, name="pck", tag="mm")
                for dt in range(DT):
                    c0 = dt * 512 + o * 128
                    nc.tensor.matmul(pp[:], akv_c[:, c0:c0 + 128],
                                     encs[dt][:],
                                     start=(dt == 0), stop=(dt == DT - 1))
                nc.scalar.activation(
                    ck_t[o][:], pp[:], AF.Identity,
                    bias=bcol(COL_QKV + l * 12 + 8 + o))
            cvaug = [build_vaug(
                kb, cvap,
                lambda dt, kb=kb: encs[dt][:, kb * 128:(kb + 1) * 128],
                lambda dt, o: akv_c[:, dt * 512 + 256 + o * 128:
                                    dt * 512 + 256 + o * 128 + 128])
                for kb in range(4)]

            wo_c_t = wop.tile([128, 2 * D], BF16, name="woc")
            nc.sync.dma_start(wo_c_t[:], wo_c[l])
            w1t = w1pool.tile([128, DT * FFL], BF16, name="w1t")
            nc.sync.dma_start(w1t[:], w1p[l])
            w2t = w2pool.tile([128, HT * D], BF16, name="w2t")
            nc.sync.dma_start(w2t[:], w2p[l])
            arc = []
            for ch in range(CH):
                boundary(ch, ars[ch], 3 * l)
                zsrc = lambda dt, ch=ch: zb[dt][:, hc(ch)]
                q_c = proj2(zsrc,
                            lambda dt, o: aq_c[:, dt * 256 + o * 128:
                                               dt * 256 + o * 128 + 128],
                            COL_QKV + l * 12 + 6, "qc", qp)
                hd = attn_core(q_c, ck_t, cvaug, ch, False,
                               COL_QKV + l * 12 + 10)
                arc.append(partial_ar(hd, wo_c_t, "wo", 2))

            # FFN per half-chunk
            arf = []
            for ch in range(CH):
                boundary(ch, arc[ch], 3 * l + 1)
                hts = []
                for ht in range(HT):
                    ppf = ps.tile([128, TL], F32, name="pf", tag="mm")
                    pp = ppf[:, 0:HCW]
                    for dt in range(DT):
                        c0 = dt * FFL + ht * 128
                        nc.tensor.matmul(pp[:], w1t[:, c0:c0 + 128],
                                         zb[dt][:, hc(ch)],
                                         start=(dt == 0), stop=(dt == DT - 1))
                    htile = hfp.tile([128, HCW], BF16, name=f"hf{ht}")
                    nc.scalar.activation(htile[:], pp[:], AF.Relu,
                                         bias=bcol(COL_B1 + l * 8 + ht))
                    hts.append(htile)
                arf.append(partial_ar(hts, w2t, "ffn2", HT))
            for ch in range(CH):
                boundary(ch, arf[ch], 3 * l + 2)

        # vocab projection: first NCHUNKED slices run on half-chunk 0
        # only (early start under the final ARs), then their half 1, then
        # the remainder full-width N=512
        NCHUNKED = 24

        def vocab_slice(vs, cols, wt):
            w = cols.stop - cols.start
            ppf = ps.tile([128, TL], F32, name="pv", tag="mm")
            pp = ppf[:, 0:w]
            for dt in range(DT):
                nc.tensor.matmul(pp[:], wt[:, dt * 128:(dt + 1) * 128],
                                 zb[dt][:, cols],
                                 start=(dt == 0), stop=(dt == DT - 1))
            osb = osp.tile([128, TL], F32, name="osb")
            osl = osb[:, 0:w]
            if vs % 2 == 0:
                nc.scalar.activation(osl, pp[:], AF.Identity,
                                     bias=bcol(COL_BOUT + vs))
            else:
                nc.vector.tensor_scalar_add(osl, pp[:], bcol(COL_BOUT + vs))
            nc.sync.dma_start(logt[vs * 128:(vs + 1) * 128, cols], osl)

        for ph, (v0, v1, cols) in enumerate([
                (0, NCHUNKED, hc(0)), (NCHUNKED, VS, slice(0, TL)),
                (0, NCHUNKED, hc(1))]):
            for vs in range(v0, v1):
                wt = wvp.tile([128, D], BF16, name="wv")
                nc.sync.dma_start(wt[:], woutp[:, vs * D:(vs + 1) * D])
                vocab_slice(vs, cols, wt)
    nc.compile()
    return nc


def _host_prepare(inputs):
    f = lambda a: np.asarray(a, dtype=np.float64)
    tobf = lambda a: a.astype(ml_dtypes.bfloat16)
    seq = np.asarray(inputs["sequence"])
    emb = np.asarray(inputs["emb"], dtype=np.float32)
    pes = np.asarray(inputs["pes"], dtype=np.float32)
    enc = np.asarray(inputs["encoder_out"], dtype=np.float32)

    x0 = emb[seq] + pes[None, :, :]                   # [B, S, D]
    xts = [np.ascontiguousarray(x0[b].T.astype(np.float32))
           for b in range(B)]                         # [D, S] per batch
    encts = [np.ascontiguousarray(tobf(enc[b].T)) for b in range(B)]

    mask = (np.arange(128)[:, None] < np.arange(128)[None, :])
    maskd = np.ascontiguousarray(tobf(mask.astype(np.float32)))

    bf = ml_dtypes.bfloat16
    attw_s = np.zeros((TPW, L, 128, DT * 3 * EL), bf)
    attq_c = np.zeros((TPW, L, 128, DT * EL), bf)
    attkv_c = np.zeros((TPW, L, 128, DT * 2 * EL), bf)
    wo_s_p = np.zeros((TPW, L, 128, 2 * D), bf)
    wo_c_p = np.zeros((TPW, L, 128, 2 * D), bf)
    w1pp = np.zeros((TPW, L, 128, DT * FFL), bf)
    w2pp = np.zeros((TPW, L, 128, HT * D), bf)
    woutpp = np.zeros((TPW, 128, VS * D), bf)
    biaspp = np.zeros((TPW, 128, NBCOL), np.float32)

    def pack_kxm(w, ncols):
        kt = w.shape[0] // 128
        return w.reshape(kt, 128, ncols).transpose(1, 0, 2).reshape(
            128, kt * ncols)

    sig = np.ones(D)
    gam = np.zeros(D)
    for l in range(L):
        for which, (wq, bq, wk, bk, wv, bv, wo, bo, g, be, m, v) in enumerate([
            (inputs["wq_s"][l], inputs["bq_s"][l], inputs["wk_s"][l],
             inputs["bk_s"][l], inputs["wv_s"][l], inputs["bv_s"][l],
             inputs["wo_s"][l], inputs["bo_s"][l], inputs["g1"][l],
             inputs["be1"][l], inputs["m1"][l], inputs["v1"][l]),
            (inputs["wq_c"][l], inputs["bq_c"][l], inputs["wk_c"][l],
             inputs["bk_c"][l], inputs["wv_c"][l], inputs["bv_c"][l],
             inputs["wo_c"][l], inputs["bo_c"][l], inputs["g2"][l],
             inputs["be2"][l], inputs["m2"][l], inputs["v2"][l]),
        ]):
            wq, wk, wv = f(wq), f(wk), f(wv)          # [H, D, DH]
            bq, bk, bv = f(bq), f(bk), f(bv)          # [H, DH]
            wo, bo = f(wo), f(bo)
            for r in range(TPW):
                h0 = r * HL
                wql = wq[h0:h0 + HL].transpose(1, 0, 2).reshape(D, EL)
                wkl = wk[h0:h0 + HL].transpose(1, 0, 2).reshape(D, EL)
                wvl = wv[h0:h0 + HL].transpose(1, 0, 2).reshape(D, EL)
                bql = bq[h0:h0 + HL].reshape(EL)
                bkl = bk[h0:h0 + HL].reshape(EL)
                bvl = bv[h0:h0 + HL].reshape(EL)
                wq_eff = (sig[:, None] * wql) / 8.0
                bq_eff = (gam @ wql + bql) / 8.0
                if which == 0:
                    wk_eff = sig[:, None] * wkl
                    bk_eff = gam @ wkl + bkl
                    wv_eff = sig[:, None] * wvl
                    bv_eff = gam @ wvl + bvl
                    wcat = np.concatenate([wq_eff, wk_eff, wv_eff], axis=1)
                    attw_s[r, l] = tobf(
                        pack_kxm(wcat, 3 * EL).astype(np.float32))
                    cb = COL_QKV + l * 12
                    for o in range(2):
                        biaspp[r, :, cb + 0 + o] = \
                            bq_eff[o * 128:(o + 1) * 128]
                        biaspp[r, :, cb + 2 + o] = \
                            bk_eff[o * 128:(o + 1) * 128]
                        biaspp[r, :, cb + 4 + o] = \
                            bv_eff[o * 128:(o + 1) * 128]
                else:
                    attq_c[r, l] = tobf(
                        pack_kxm(wq_eff, EL).astype(np.float32))
                    kvcat = np.concatenate([wkl, wvl], axis=1)
                    attkv_c[r, l] = tobf(
                        pack_kxm(kvcat, 2 * EL).astype(np.float32))
                    cb = COL_QKV + l * 12
                    for o in range(2):
                        biaspp[r, :, cb + 6 + o] = \
                            bq_eff[o * 128:(o + 1) * 128]
                        biaspp[r, :, cb + 8 + o] = bkl[o * 128:(o + 1) * 128]
                        biaspp[r, :, cb + 10 + o] = bvl[o * 128:(o + 1) * 128]
                wo_loc = wo[r * EL:(r + 1) * EL, :]       # [256, 1024]
                wo_pk = wo_loc.reshape(2, 128, D).transpose(1, 0, 2).reshape(
                    128, 2 * D)
                (wo_s_p if which == 0 else wo_c_p)[r, l] = tobf(
                    wo_pk.astype(np.float32))
            bnd = 3 * l + which
            for r in range(TPW):
                for dt in range(DT):
                    biaspp[r, :, COL_SIG + bnd * 8 + dt] = \
                        sig[dt * 128:(dt + 1) * 128].astype(np.float32)
            beta = gam + bo
            s = f(g) / np.sqrt(f(v) + EPS)
            cshift = f(be) - f(m) * s
            sig = s
            gam = s * beta + cshift

        w1, b1 = f(inputs["w1"][l]), f(inputs["b1"][l])
        w2, b2 = f(inputs["w2"][l]), f(inputs["b2"][l])
        for r in range(TPW):
            cols = slice(r * FFL, (r + 1) * FFL)
            w1_eff = sig[:, None] * w1[:, cols]
            b1_eff = gam @ w1[:, cols] + b1[cols]
            w1pp[r, l] = tobf(pack_kxm(w1_eff, FFL).astype(np.float32))
            w2pp[r, l] = tobf(pack_kxm(w2[cols, :], D).astype(np.float32))
            for ht in range(HT):
                biaspp[r, :, COL_B1 + l * 8 + ht] = \
                    b1_eff[ht * 128:(ht + 1) * 128].astype(np.float32)
        bnd = 3 * l + 2
        for r in range(TPW):
            for dt in range(DT):
                biaspp[r, :, COL_SIG + bnd * 8 + dt] = \
                    sig[dt * 128:(dt + 1) * 128].astype(np.float32)
        beta = gam + b2
        s = f(inputs["g3"][l]) / np.sqrt(f(inputs["v3"][l]) + EPS)
        cshift = f(inputs["be3"][l]) - f(inputs["m3"][l]) * s
        sig = s
        gam = s * beta + cshift

    wout, bout = f(inputs["w_out"]), f(inputs["b_out"])
    for r in range(TPW):
        wsl = np.zeros((D, VPAD))
        bsl = np.zeros(VPAD)
        cols = slice(r * VL, (r + 1) * VL)
        wsl[:, :VL] = wout[:, cols]
        bsl[:VL] = bout[cols]
        wout_eff = sig[:, None] * wsl
        bout_eff = gam @ wsl + bsl
        woutpp[r] = tobf(wout_eff.reshape(DT, 128, VS, 128).transpose(
            1, 2, 0, 3).reshape(128, VS * D).astype(np.float32))
        for vs in range(VS):
            biaspp[r, :, COL_BOUT + vs] = \
                bout_eff[vs * 128:(vs + 1) * 128].astype(np.float32)

    biaspp[:, :, COL_EPS] = 1e-30
    in_maps = []
    for c in range(NC):
        g, r = c // TPW, c % TPW
        in_maps.append({
            "xt": xts[g], "enct": encts[g],
            "attw_s": attw_s[r], "attq_c": attq_c[r], "attkv_c": attkv_c[r],
            "wo_s": wo_s_p[r], "wo_c": wo_c_p[r],
            "w1p": w1pp[r], "w2p": w2pp[r], "woutp": woutpp[r],
            "biasp": biaspp[r], "maskd": maskd,
            "identd": tobf(np.eye(128, dtype=np.float32)),
            "onesd": np.ones((128, 64), dtype=ml_dtypes.bfloat16),
        })
    return in_maps


_NC_CACHE = {}


def _get_program():
    if "nc" not in _NC_CACHE:
        _NC_CACHE["nc"] = _build_program()
    return _NC_CACHE["nc"]


def run(inputs, trace=False):
    nc = _get_program()
    in_maps = _host_prepare(inputs)
    res = bass_utils.run_bass_kernel_spmd(nc, in_maps, list(range(NC)),
                                          trace=trace)
    out = np.empty((B, S, V), np.float32)
    for c in range(NC):
        g, r = c // TPW, c % TPW
        out[g, :, r * VL:(r + 1) * VL] = res.results[c]["logt"][:VL, :].T
    return out, res


def kernel(**inputs):
    out, _ = run(inputs)
    return out


# revision 6
# speedup vs baseline: 1.0335x; 1.0120x over previous
"""Trainium2 Bass kernel, v2: TP4 x DP2 for the 4-layer decoder.

  - Cores 0-3 handle batch 0, cores 4-7 batch 1 (disjoint 4-rank
    replica groups -> the two groups' collectives run concurrently).
  - Within a group: tensor-parallel over 4 heads/core (EL=256),
    FFN 1024 hidden/core, vocab 8000 cols/core.
  - Each batch's 512 tokens split into two 256-token half-chunks;
    boundary AllReduces are per-half (512KB bf16, 4-rank Mesh) so the
    PE computes one half while the other half's AR is in flight.
  - All matmuls bf16 (residual z kept fp32 on SBUF with a bf16 shadow
    zb used as matmul rhs).  BN + biases folded host-side as before.
"""

import sys
import numpy as np

if "/opt/trn_rl_repo" not in sys.path:
    sys.path.insert(0, "/opt/trn_rl_repo")

import ml_dtypes
import concourse.bass as bass
import concourse.mybir as mybir
import concourse.tile as tile
from concourse import bacc
from concourse import bass_utils

V, D, H, L, B, S, SE = 32000, 1024, 16, 4, 2, 512, 512
DH = D // H            # 64
EPS = 1e-3
NC = 8
TPW = 4                # tensor-parallel width (per group)
HL = H // TPW          # 4 heads per core
EL = HL * DH           # 256 local head dims
FF = 4 * D
FFL = FF // TPW        # 1024 ffn hidden per core
VL = V // TPW          # 8000
VPAD = 8192
VS = VPAD // 128       # 64 vocab slices
TL = S                 # 512 tokens per group (one batch)
CH = 2                 # half-chunks
HCW = TL // CH         # 256
DT = D // 128          # 8
HT = FFL // 128        # 8
NBND = 3 * L           # 12

F32 = mybir.dt.float32
BF16 = mybir.dt.bfloat16
AF = mybir.ActivationFunctionType
OP = mybir.AluOpType

RG = [[0, 1, 2, 3], [4, 5, 6, 7]]
_CCCNT = [0]

# bias-tile column layout
COL_QKV = 0                      # L*12: l*12 + proj(6)*2 + o
COL_B1 = COL_QKV + 12 * L        # L*8: l*8 + ht
COL_SIG = COL_B1 + 8 * L         # 12*8: bnd*8 + dt
COL_BOUT = COL_SIG + 8 * NBND    # 64
COL_EPS = COL_BOUT + VS
NBCOL = COL_EPS + 1


def _build_program():
    nc = bacc.Bacc("TRN2", target_bir_lowering=False, debug=False,
                   num_devices=NC)
    dd = lambda name, shape, dtype=BF16, kind="ExternalInput": \
        nc.dram_tensor(name, shape, dtype, kind=kind).ap()

    xt = dd("xt", [D, TL], F32)
    enct = dd("enct", [D, TL])
    attw_s = dd("attw_s", [L, 128, DT * 3 * EL])    # col = dt*768+p*256+o*128
    attq_c = dd("attq_c", [L, 128, DT * EL])        # col = dt*256 + o*128
    attkv_c = dd("attkv_c", [L, 128, DT * 2 * EL])  # col = dt*512+{k0,v256}+o*128
    wo_s = dd("wo_s", [L, 128, 2 * D])              # col = kt*1024 + dout
    wo_c = dd("wo_c", [L, 128, 2 * D])
    w1p = dd("w1p", [L, 128, DT * FFL])             # col = dt*1024 + f
    w2p = dd("w2p", [L, 128, HT * D])               # col = ht*1024 + dout
    woutp = dd("woutp", [128, VS * D])              # col = vs*1024 + dt*128 + j
    biasp = dd("biasp", [128, NBCOL], F32)
    maskd = dd("maskd", [128, 128])                 # strictly-lower 0/1
    identd = dd("identd", [128, 128])
    onesd = dd("onesd", [128, 64])
    logt = dd("logt", [VPAD, TL], F32, kind="ExternalOutput")

    from contextlib import ExitStack
    with tile.TileContext(nc) as tc, ExitStack() as _es:
        P = lambda **kw: _es.enter_context(tc.tile_pool(**kw))
        cst = P(name="cst", bufs=1)
        zp = P(name="zp", bufs=1)
        zbp = P(name="zbp", bufs=1)
        encp = P(name="encp", bufs=1)
        qp = P(name="qp", bufs=2)
        kvp = P(name="kvp", bufs=2)
        ckvp = P(name="ckvp", bufs=2)
        vap = P(name="vap", bufs=2)
        cvap = P(name="cvap", bufs=2)
        esp = P(name="esp", bufs=5)
        hdp = P(name="hdp", bufs=2)
        csp = P(name="csp", bufs=2)
        hfp = P(name="hfp", bufs=2)
        arp = P(name="arp", bufs=2)
        aop = P(name="aop", bufs=2)
        wap = P(name="wap", bufs=2)
        waqc = P(name="waqc", bufs=1)
        wakv = P(name="wakv", bufs=1)
        wop = P(name="wop", bufs=1)
        w1pool = P(name="w1pool", bufs=1)
        w2pool = P(name="w2pool", bufs=1)
        wvp = P(name="wvp", bufs=16)
        osp = P(name="osp", bufs=3)
        ps = P(name="ps", bufs=8, space="PSUM")
        dram = P(name="dram", bufs=4, space="DRAM")

        bias_sb = cst.tile([128, NBCOL], F32)
        nc.sync.dma_start(bias_sb[:], biasp[:])
        mask_sb = cst.tile([128, 128], BF16)
        nc.sync.dma_start(mask_sb[:], maskd[:])
        ident = cst.tile([128, 128], BF16)
        nc.sync.dma_start(ident[:], identd[:])
        ones_sb = cst.tile([128, 64], BF16)
        nc.sync.dma_start(ones_sb[:], onesd[:])
        zeros_sb = cst.tile([128, 256], BF16)
        nc.vector.tensor_scalar_mul(zeros_sb[:, 0:64], ones_sb[:], 0.0)
        nc.vector.tensor_copy(zeros_sb[:, 64:128], zeros_sb[:, 0:64])
        nc.vector.tensor_copy(zeros_sb[:, 128:256], zeros_sb[:, 0:128])

        z = [None] * DT
        zb = [None] * DT
        for dt in range(DT):
            zt = zp.tile([128, TL], F32, name=f"z{dt}")
            nc.sync.dma_start(zt[:], xt[dt * 128:(dt + 1) * 128, :])
            z[dt] = zt
            zbt = zbp.tile([128, TL], BF16, name=f"zb{dt}")
            if dt % 2 == 0:
                nc.scalar.activation(zbt[:], zt[:], AF.Copy)
            else:
                nc.vector.tensor_copy(zbt[:], zt[:])
            zb[dt] = zbt

        # encoder activations loaded after z (needed only at cross-attn)
        encs = []
        for dt in range(DT):
            et = encp.tile([128, TL], BF16, name=f"enc{dt}")
            nc.sync.dma_start(et[:], enct[dt * 128:(dt + 1) * 128, :])
            encs.append(et)

        def bcol(c):
            return bias_sb[:, c:c + 1]

        def hc(ch):
            return slice(ch * HCW, (ch + 1) * HCW)

        def build_vaug(kb, pool, vsrc, vwsl):
            """Token-major vaug [128, 384] built straight from the v
            projection: partitions = tokens of key-block kb, cols =
            [h0|ones|h1|h2|ones|h3].  vsrc(dt) is the [128, 128] lhsT
            slice of the activation source for this kb; vwsl(dt, o) the
            [128, 128] v-weight slice for feature half o."""
            va = pool.tile([128, 384], BF16, name=f"va{kb}")
            for o in range(2):
                ppf = ps.tile([128, TL], F32, name="pvt", tag="mm")
                pp = ppf[:, 0:128]
                for dt in range(DT):
                    nc.tensor.matmul(pp[:], vsrc(dt), vwsl(dt, o),
                                     start=(dt == 0), stop=(dt == DT - 1))
                base = 192 * o
                nc.scalar.activation(va[:, base:base + 64], pp[:, 0:64],
                                     AF.Identity)
                dsth1 = va[:, 128:192] if o == 0 else va[:, 320:384]
                nc.scalar.activation(dsth1, pp[:, 64:128], AF.Identity)
            nc.vector.tensor_copy(va[:, 64:128], ones_sb[:])
            nc.vector.tensor_copy(va[:, 256:320], ones_sb[:])
            return va

        def attn_core(qt, kt, vaug, ch, causal, bvc):
            """qt: 2x[128,HCW]; kt: 2x[128,TL]; vaug: list of [128,384].
            bvc: bias column base for the deferred v bias (+o).
            Returns hd: 2 tiles [128, HCW] bf16."""
            nkb = 2 * (ch + 1) if causal else 4
            hd = [hdp.tile([128, HCW], BF16, name=f"hd{o}") for o in range(2)]
            for h in range(HL):
                o, ro = h // 2, (h % 2) * 64
                es = []
                for kb in range(nkb):
                    q0 = max(0, kb * 128 - ch * HCW) if causal else 0
                    ppf = ps.tile([128, TL], F32, name="psc", tag="mm")
                    pp = ppf[:, 0:HCW]
                    nc.tensor.matmul(pp[:],
                                     kt[o][ro:ro + 64,
                                           kb * 128:(kb + 1) * 128],
                                     qt[o][ro:ro + 64, :],
                                     start=True, stop=True)
                    et = esp.tile([128, HCW], BF16, name="es")
                    nc.scalar.activation(et[:, q0:HCW], pp[:, q0:HCW], AF.Exp)
                    if causal and kb * 128 >= ch * HCW:
                        if q0 > 0:
                            nc.vector.tensor_copy(et[:, 0:q0],
                                                  zeros_sb[:, 0:q0])
                        nc.vector.tensor_tensor(et[:, q0:q0 + 128],
                                                et[:, q0:q0 + 128],
                                                mask_sb[:], op=OP.mult)
                    es.append(et)
                pof = ps.tile([128, TL], F32, name="po", tag="mm")
                po = pof[:, 0:HCW]
                # stationary window: [val|ones] for even h, [ones|val] odd
                ws = 64 * h + (64 if h >= 2 else 0)
                for kb in range(nkb):
                    nc.tensor.matmul(po[:], vaug[kb][:, ws:ws + 128],
                                     es[kb][:],
                                     start=(kb == 0), stop=(kb == nkb - 1))
                if h % 2 == 0:
                    nrows, crows = po[0:64, :], po[64:128, :]
                else:
                    nrows, crows = po[64:128, :], po[0:64, :]
                cs = csp.tile([64, HCW], F32, name="cs")
                nc.scalar.activation(cs[:], crows, AF.Identity,
                                     bias=bias_sb[0:64, COL_EPS:COL_EPS + 1])
                rc = csp.tile([64, HCW], F32, name="rc")
                nc.vector.reciprocal_approx_fast(out=rc[:], in_=cs[:])
                cs2 = csp.tile([64, HCW], F32, name="cs2")
                nc.vector.tensor_tensor(cs2[:], nrows, rc[:], op=OP.mult)
                nc.scalar.activation(
                    hd[o][ro:ro + 64, :], cs2[:], AF.Identity,
                    bias=bias_sb[ro:ro + 64, bvc + o:bvc + o + 1])
            if causal and ch == 0:
                # position 0 attends to nothing -> exactly zero output
                for o in range(2):
                    nc.vector.tensor_scalar_mul(hd[o][:, 0:1],
                                                hd[o][:, 0:1], 0.0)
            return hd

        def partial_ar(src, wsel, mode, nsrc):
            """src: list of tiles [128, HCW] (contraction tiles);
            wsel cols: i*1024 + dout*128.  AllReduce [D, HCW] bf16."""
            arin = dram.tile([D, HCW], BF16, name="arin")
            arout = dram.tile([D, HCW], BF16, name="arout")
            ocw = aop.tile([128, DT * HCW], BF16, name="ocw")
            for half in range(2):
                for dout in range(half * 4, half * 4 + 4):
                    pwf = ps.tile([128, TL], F32, name="pw", tag="mm")
                    pw = pwf[:, 0:HCW]
                    for i in range(nsrc):
                        c0 = i * D + dout * 128
                        nc.tensor.matmul(pw[:], wsel[:, c0:c0 + 128],
                                         src[i][:],
                                         start=(i == 0), stop=(i == nsrc - 1))
                    osl = ocw[:, dout * HCW:(dout + 1) * HCW]
                    if dout % 2 == 0:
                        nc.scalar.activation(osl, pw[:], AF.Copy)
                    else:
                        nc.vector.tensor_copy(osl, pw[:])
                h0 = half * 4
                nc.sync.dma_start(
                    arin[h0 * 128:(h0 + 4) * 128, :].rearrange(
                        "(dt p) t -> p dt t", p=128),
                    ocw[:, h0 * HCW:(h0 + 4) * HCW].rearrange(
                        "p (dt t) -> p dt t", t=HCW))
            import os as _os
            nlim = int(_os.environ.get("K2_CC_N", "999"))
            _CCCNT[0] += 1
            if _os.environ.get("K2_SKIP_CC") == "1" or _CCCNT[0] > nlim:
                return arin
            rg = [[0, 1, 2, 3, 4, 5, 6, 7]] \
                if _os.environ.get("K2_RG8") == "1" else RG
            nc.gpsimd.collective_compute("AllReduce", OP.add,
                                         replica_groups=rg,
                                         ins=[arin[:]], outs=[arout[:]])
            return arout

        def boundary(ch, arout, bnd):
            art = arp.tile([128, DT * HCW], BF16, name="art")
            for half in range(2):
                h0 = half * 4
                nc.sync.dma_start(
                    art[:, h0 * HCW:(h0 + 4) * HCW].rearrange(
                        "p (dt t) -> p dt t", t=HCW),
                    arout[h0 * 128:(h0 + 4) * 128, :].rearrange(
                        "(dt p) t -> p dt t", p=128))
            for dt in range(DT):
                # bf16 shadow on the critical path (DVE), fp32 residual
                # update off-path on the otherwise-idle GpSimd engine
                nc.vector.scalar_tensor_tensor(
                    zb[dt][:, hc(ch)], z[dt][:, hc(ch)],
                    bcol(COL_SIG + bnd * 8 + dt),
                    art[:, dt * HCW:(dt + 1) * HCW], OP.mult, OP.add)
                nc.vector.scalar_tensor_tensor(
                    z[dt][:, hc(ch)], z[dt][:, hc(ch)],
                    bcol(COL_SIG + bnd * 8 + dt),
                    art[:, dt * HCW:(dt + 1) * HCW], OP.mult, OP.add)

        def proj2(srcs, wsl, bc0, nm, pool):
            """2 out-tiles [128, HCW] = w.T @ src (+bias)."""
            outs = []
            for o in range(2):
                ppf = ps.tile([128, TL], F32, name="pp", tag="mm")
                pp = ppf[:, 0:HCW]
                for dt in range(DT):
                    nc.tensor.matmul(pp[:], wsl(dt, o), srcs(dt),
                                     start=(dt == 0), stop=(dt == DT - 1))
                pt = pool.tile([128, HCW], BF16, name=f"{nm}{o}")
                nc.scalar.activation(pt[:], pp[:], AF.Identity,
                                     bias=bcol(bc0 + o))
                outs.append(pt)
            return outs

        for l in range(L):
            aw_s = wap.tile([128, DT * 3 * EL], BF16, name="aw")
            nc.sync.dma_start(aw_s[:], attw_s[l])
            wo_s_t = wop.tile([128, 2 * D], BF16, name="wot")
            nc.sync.dma_start(wo_s_t[:], wo_s[l])
            aq_c = waqc.tile([128, DT * EL], BF16, name="aqc")
            nc.sync.dma_start(aq_c[:], attq_c[l])
            akv_c = wakv.tile([128, DT * 2 * EL], BF16, name="akvc")
            nc.sync.dma_start(akv_c[:], attkv_c[l])

            # self attention per half-chunk
            k_t = [kvp.tile([128, TL], BF16, name=f"kt{o}") for o in range(2)]
            vaug = [None] * 4
            ars = []
            for ch in range(CH):
                q_t = []
                for o in range(2):
                    ppf = ps.tile([128, TL], F32, name="pq", tag="mm")
                    pp = ppf[:, 0:HCW]
                    for dt in range(DT):
                        c0 = dt * 768 + 0 * 256 + o * 128
                        nc.tensor.matmul(pp[:], aw_s[:, c0:c0 + 128],
                                         zb[dt][:, hc(ch)],
                                         start=(dt == 0), stop=(dt == DT - 1))
                    qt = qp.tile([128, HCW], BF16, name=f"qt{o}")
                    nc.scalar.activation(qt[:], pp[:], AF.Identity,
                                         bias=bcol(COL_QKV + l * 12 + o))
                    q_t.append(qt)
                for o in range(2):
                    ppf = ps.tile([128, TL], F32, name="pkv", tag="mm")
                    pp = ppf[:, 0:HCW]
                    for dt in range(DT):
                        c0 = dt * 768 + 256 + o * 128
                        nc.tensor.matmul(pp[:], aw_s[:, c0:c0 + 128],
                                         zb[dt][:, hc(ch)],
                                         start=(dt == 0),
                                         stop=(dt == DT - 1))
                    nc.scalar.activation(
                        k_t[o][:, hc(ch)], pp[:], AF.Identity,
                        bias=bcol(COL_QKV + l * 12 + 2 + o))
                for kb in range(2 * ch, 2 * ch + 2):
                    vaug[kb] = build_vaug(
                        kb, vap,
                        lambda dt, kb=kb: zb[dt][:, kb * 128:(kb + 1) * 128],
                        lambda dt, o: aw_s[:, dt * 768 + 512 + o * 128:
                                           dt * 768 + 512 + o * 128 + 128])
                hd = attn_core(q_t, k_t, vaug, ch, True,
                               COL_QKV + l * 12 + 4)
                ars.append(partial_ar(hd, wo_s_t, "wo", 2))

            # hoisted cross k/v (full 512 tokens, encoder-sourced)
            ck_t = [ckvp.tile([128, TL], BF16, name=f"ck{o}")
                    for o in range(2)]
            for o in range(2):
                pp = ps.tile([128, TL], F32, name="pck", tag="mm")
                for dt in range(DT):
                    c0 = dt * 512 + o * 128
                    nc.tensor.matmul(pp[:], akv_c[:, c0:c0 + 128],
                                     encs[dt][:],
                                     start=(dt == 0), stop=(dt == DT - 1))
                nc.scalar.activation(
                    ck_t[o][:], pp[:], AF.Identity,
                    bias=bcol(COL_QKV + l * 12 + 8 + o))
            cvaug = [build_vaug(
                kb, cvap,
                lambda dt, kb=kb: encs[dt][:, kb * 128:(kb + 1) * 128],
                lambda dt, o: akv_c[:, dt * 512 + 256 + o * 128:
                                    dt * 512 + 256 + o * 128 + 128])
                for kb in range(4)]

            wo_c_t = wop.tile([128, 2 * D], BF16, name="woc")
            nc.sync.dma_start(wo_c_t[:], wo_c[l])
            arc = []
            for ch in range(CH):
                boundary(ch, ars[ch], 3 * l)
                zsrc = lambda dt, ch=ch: zb[dt][:, hc(ch)]
                q_c = proj2(zsrc,
                            lambda dt, o: aq_c[:, dt * 256 + o * 128:
                                               dt * 256 + o * 128 + 128],
                            COL_QKV + l * 12 + 6, "qc", qp)
                hd = attn_core(q_c, ck_t, cvaug, ch, False,
                               COL_QKV + l * 12 + 10)
                arc.append(partial_ar(hd, wo_c_t, "wo", 2))
                if ch == 0:
                    # ffn weights fetched away from the self-attn ARs
                    w1t = w1pool.tile([128, DT * FFL], BF16, name="w1t")
                    nc.sync.dma_start(w1t[:], w1p[l])
                    w2t = w2pool.tile([128, HT * D], BF16, name="w2t")
                    nc.sync.dma_start(w2t[:], w2p[l])

            # FFN per half-chunk
            arf = []
            for ch in range(CH):
                boundary(ch, arc[ch], 3 * l + 1)
                hts = []
                for ht in range(HT):
                    ppf = ps.tile([128, TL], F32, name="pf", tag="mm")
                    pp = ppf[:, 0:HCW]
                    for dt in range(DT):
                        c0 = dt * FFL + ht * 128
                        nc.tensor.matmul(pp[:], w1t[:, c0:c0 + 128],
                                         zb[dt][:, hc(ch)],
                                         start=(dt == 0), stop=(dt == DT - 1))
                    htile = hfp.tile([128, HCW], BF16, name=f"hf{ht}")
                    nc.scalar.activation(htile[:], pp[:], AF.Relu,
                                         bias=bcol(COL_B1 + l * 8 + ht))
                    hts.append(htile)
                arf.append(partial_ar(hts, w2t, "ffn2", HT))
            for ch in range(CH):
                boundary(ch, arf[ch], 3 * l + 2)

        # vocab projection: first NCHUNKED slices run on half-chunk 0
        # only (early start under the final ARs), then their half 1, then
        # the remainder full-width N=512
        NCHUNKED = 24

        def vocab_slice(vs, cols, wt):
            w = cols.stop - cols.start
            ppf = ps.tile([128, TL], F32, name="pv", tag="mm")
            pp = ppf[:, 0:w]
            for dt in range(DT):
                nc.tensor.matmul(pp[:], wt[:, dt * 128:(dt + 1) * 128],
                                 zb[dt][:, cols],
                                 start=(dt == 0), stop=(dt == DT - 1))
            osb = osp.tile([128, TL], F32, name="osb")
            osl = osb[:, 0:w]
            if vs % 2 == 0:
                nc.scalar.activation(osl, pp[:], AF.Identity,
                                     bias=bcol(COL_BOUT + vs))
            else:
                nc.vector.tensor_scalar_add(osl, pp[:], bcol(COL_BOUT + vs))
            nc.sync.dma_start(logt[vs * 128:(vs + 1) * 128, cols], osl)

        for ph, (v0, v1, cols) in enumerate([
                (0, NCHUNKED, hc(0)), (NCHUNKED, VS, slice(0, TL)),
                (0, NCHUNKED, hc(1))]):
            for vs in range(v0, v1):
                wt = wvp.tile([128, D], BF16, name="wv")
                nc.sync.dma_start(wt[:], woutp[:, vs * D:(vs + 1) * D])
                vocab_slice(vs, cols, wt)
    nc.compile()
    return nc


def _host_prepare(inputs):
    f = lambda a: np.asarray(a, dtype=np.float64)
    tobf = lambda a: a.astype(ml_dtypes.bfloat16)
    seq = np.asarray(inputs["sequence"])
    emb = np.asarray(inputs["emb"], dtype=np.float32)
    pes = np.asarray(inputs["pes"], dtype=np.float32)
    enc = np.asarray(inputs["encoder_out"], dtype=np.float32)

    x0 = emb[seq] + pes[None, :, :]                   # [B, S, D]
    xts = [np.ascontiguousarray(x0[b].T.astype(np.float32))
           for b in range(B)]                         # [D, S] per batch
    encts = [np.ascontiguousarray(tobf(enc[b].T)) for b in range(B)]

    mask = (np.arange(128)[:, None] < np.arange(128)[None, :])
    maskd = np.ascontiguousarray(tobf(mask.astype(np.float32)))

    bf = ml_dtypes.bfloat16
    attw_s = np.zeros((TPW, L, 128, DT * 3 * EL), bf)
    attq_c = np.zeros((TPW, L, 128, DT * EL), bf)
    attkv_c = np.zeros((TPW, L, 128, DT * 2 * EL), bf)
    wo_s_p = np.zeros((TPW, L, 128, 2 * D), bf)
    wo_c_p = np.zeros((TPW, L, 128, 2 * D), bf)
    w1pp = np.zeros((TPW, L, 128, DT * FFL), bf)
    w2pp = np.zeros((TPW, L, 128, HT * D), bf)
    woutpp = np.zeros((TPW, 128, VS * D), bf)
    biaspp = np.zeros((TPW, 128, NBCOL), np.float32)

    def pack_kxm(w, ncols):
        kt = w.shape[0] // 128
        return w.reshape(kt, 128, ncols).transpose(1, 0, 2).reshape(
            128, kt * ncols)

    sig = np.ones(D)
    gam = np.zeros(D)
    for l in range(L):
        for which, (wq, bq, wk, bk, wv, bv, wo, bo, g, be, m, v) in enumerate([
            (inputs["wq_s"][l], inputs["bq_s"][l], inputs["wk_s"][l],
             inputs["bk_s"][l], inputs["wv_s"][l], inputs["bv_s"][l],
             inputs["wo_s"][l], inputs["bo_s"][l], inputs["g1"][l],
             inputs["be1"][l], inputs["m1"][l], inputs["v1"][l]),
            (inputs["wq_c"][l], inputs["bq_c"][l], inputs["wk_c"][l],
             inputs["bk_c"][l], inputs["wv_c"][l], inputs["bv_c"][l],
             inputs["wo_c"][l], inputs["bo_c"][l], inputs["g2"][l],
             inputs["be2"][l], inputs["m2"][l], inputs["v2"][l]),
        ]):
            wq, wk, wv = f(wq), f(wk), f(wv)          # [H, D, DH]
            bq, bk, bv = f(bq), f(bk), f(bv)          # [H, DH]
            wo, bo = f(wo), f(bo)
            for r in range(TPW):
                h0 = r * HL
                wql = wq[h0:h0 + HL].transpose(1, 0, 2).reshape(D, EL)
                wkl = wk[h0:h0 + HL].transpose(1, 0, 2).reshape(D, EL)
                wvl = wv[h0:h0 + HL].transpose(1, 0, 2).reshape(D, EL)
                bql = bq[h0:h0 + HL].reshape(EL)
                bkl = bk[h0:h0 + HL].reshape(EL)
                bvl = bv[h0:h0 + HL].reshape(EL)
                wq_eff = (sig[:, None] * wql) / 8.0
                bq_eff = (gam @ wql + bql) / 8.0
                if which == 0:
                    wk_eff = sig[:, None] * wkl
                    bk_eff = gam @ wkl + bkl
                    wv_eff = sig[:, None] * wvl
                    bv_eff = gam @ wvl + bvl
                    wcat = np.concatenate([wq_eff, wk_eff, wv_eff], axis=1)
                    attw_s[r, l] = tobf(
                        pack_kxm(wcat, 3 * EL).astype(np.float32))
                    cb = COL_QKV + l * 12
                    for o in range(2):
                        biaspp[r, :, cb + 0 + o] = \
                            bq_eff[o * 128:(o + 1) * 128]
                        biaspp[r, :, cb + 2 + o] = \
                            bk_eff[o * 128:(o + 1) * 128]
                        biaspp[r, :, cb + 4 + o] = \
                            bv_eff[o * 128:(o + 1) * 128]
                else:
                    attq_c[r, l] = tobf(
                        pack_kxm(wq_eff, EL).astype(np.float32))
                    kvcat = np.concatenate([wkl, wvl], axis=1)
                    attkv_c[r, l] = tobf(
                        pack_kxm(kvcat, 2 * EL).astype(np.float32))
                    cb = COL_QKV + l * 12
                    for o in range(2):
                        biaspp[r, :, cb + 6 + o] = \
                            bq_eff[o * 128:(o + 1) * 128]
                        biaspp[r, :, cb + 8 + o] = bkl[o * 128:(o + 1) * 128]
                        biaspp[r, :, cb + 10 + o] = bvl[o * 128:(o + 1) * 128]
                wo_loc = wo[r * EL:(r + 1) * EL, :]       # [256, 1024]
                wo_pk = wo_loc.reshape(2, 128, D).transpose(1, 0, 2).reshape(
                    128, 2 * D)
                (wo_s_p if which == 0 else wo_c_p)[r, l] = tobf(
                    wo_pk.astype(np.float32))
            bnd = 3 * l + which
            for r in range(TPW):
                for dt in range(DT):
                    biaspp[r, :, COL_SIG + bnd * 8 + dt] = \
                        sig[dt * 128:(dt + 1) * 128].astype(np.float32)
            beta = gam + bo
            s = f(g) / np.sqrt(f(v) + EPS)
            cshift = f(be) - f(m) * s
            sig = s
            gam = s * beta + cshift

        w1, b1 = f(inputs["w1"][l]), f(inputs["b1"][l])
        w2, b2 = f(inputs["w2"][l]), f(inputs["b2"][l])
        for r in range(TPW):
            cols = slice(r * FFL, (r + 1) * FFL)
            w1_eff = sig[:, None] * w1[:, cols]
            b1_eff = gam @ w1[:, cols] + b1[cols]
            w1pp[r, l] = tobf(pack_kxm(w1_eff, FFL).astype(np.float32))
            w2pp[r, l] = tobf(pack_kxm(w2[cols, :], D).astype(np.float32))
            for ht in range(HT):
                biaspp[r, :, COL_B1 + l * 8 + ht] = \
                    b1_eff[ht * 128:(ht + 1) * 128].astype(np.float32)
        bnd = 3 * l + 2
        for r in range(TPW):
            for dt in range(DT):
                biaspp[r, :, COL_SIG + bnd * 8 + dt] = \
                    sig[dt * 128:(dt + 1) * 128].astype(np.float32)
        beta = gam + b2
        s = f(inputs["g3"][l]) / np.sqrt(f(inputs["v3"][l]) + EPS)
        cshift = f(inputs["be3"][l]) - f(inputs["m3"][l]) * s
        sig = s
        gam = s * beta + cshift

    wout, bout = f(inputs["w_out"]), f(inputs["b_out"])
    for r in range(TPW):
        wsl = np.zeros((D, VPAD))
        bsl = np.zeros(VPAD)
        cols = slice(r * VL, (r + 1) * VL)
        wsl[:, :VL] = wout[:, cols]
        bsl[:VL] = bout[cols]
        wout_eff = sig[:, None] * wsl
        bout_eff = gam @ wsl + bsl
        woutpp[r] = tobf(wout_eff.reshape(DT, 128, VS, 128).transpose(
            1, 2, 0, 3).reshape(128, VS * D).astype(np.float32))
        for vs in range(VS):
            biaspp[r, :, COL_BOUT + vs] = \
                bout_eff[vs * 128:(vs + 1) * 128].astype(np.float32)

    biaspp[:, :, COL_EPS] = 1e-30
    in_maps = []
    for c in range(NC):
        g, r = c // TPW, c % TPW
        in_maps.append({
            "xt": xts[g], "enct": encts[g],
            "attw_s": attw_s[r], "attq_c": attq_c[r], "attkv_c": attkv_c[r],
            "wo_s": wo_s_p[r], "wo_c": wo_c_p[r],
            "w1p": w1pp[r], "w2p": w2pp[r], "woutp": woutpp[r],
            "biasp": biaspp[r], "maskd": maskd,
            "identd": tobf(np.eye(128, dtype=np.float32)),
            "onesd": np.ones((128, 64), dtype=ml_dtypes.bfloat16),
        })
    return in_maps


_NC_CACHE = {}


def _get_program():
    if "nc" not in _NC_CACHE:
        _NC_CACHE["nc"] = _build_program()
    return _NC_CACHE["nc"]


def run(inputs, trace=False):
    nc = _get_program()
    in_maps = _host_prepare(inputs)
    res = bass_utils.run_bass_kernel_spmd(nc, in_maps, list(range(NC)),
                                          trace=trace)
    out = np.empty((B, S, V), np.float32)
    for c in range(NC):
        g, r = c // TPW, c % TPW
        out[g, :, r * VL:(r + 1) * VL] = res.results[c]["logt"][:VL, :].T
    return out, res


def kernel(**inputs):
    out, _ = run(inputs)
    return out
